# revision 1
# baseline (speedup 1.0000x reference)
"""Trainium2 Bass kernel for nn_Attention: single-head attention,
B=32, N=1024, DIM=512, fp32.

    q = X @ Wq.T ; k = X @ Wk.T ; v = X @ Wv.T
    out = softmax((q k^T)/sqrt(D)) @ v

Strategy (8 NeuronCores, data-parallel over batch, 4 batches/core):
  - Host folds A = (Wq.T @ Wk)/sqrt(D)  so scores = X A X.T  — saves one
    projection-sized matmul per batch and needs only X (transposed) on
    device.
  - All tensors live transposed on device: XT [d, n], GT = (X A).T,
    V [n, e], scores ST [k, q] (k on partitions).  Softmax runs along
    the partition axis: exp on ScalarE, partition sums via a ones-vector
    matmul, broadcast of 1/denom via a rank-1 ones matmul, normalization
    fused into the PSUM->SBUF eviction on VectorE.  Attention output is
    produced transposed (OT [e, q]) and the host transposes it back.
  - Matmuls use the float32r PE mode (full-rate fp32 streaming).
"""
import numpy as np

B, N, D = 32, 1024, 512
NCORES = 8
BPC = B // NCORES          # batches per core
DC = D // 128              # 4 chunks of 128 along d / e
KC = N // 128              # 8 chunks of 128 along k
QH = N // 512              # 2 q-halves of 512

_cache = {}


def _split_sync_waits(nc):
    """walrus on this image accepts at most ONE semaphore wait per
    instruction; hoist extras onto InstNoOp carriers on the same engine
    (same-engine program order preserves the gating)."""
    import concourse.mybir as mybir

    ctr = 0
    for f in nc.m.functions:
        for bb in f.blocks:
            out = []
            changed = False
            for ins in bb.instructions:
                si = getattr(ins, "sync_info", None)
                waits = list(si.on_wait) if si and si.on_wait else []
                if len(waits) > 1:
                    for w in waits[:-1]:
                        ctr += 1
                        out.append(
                            mybir.InstNoOp(
                                name=f"wsplit-{ctr}",
                                engine=ins.engine,
                                bass_nofuse=True,
                                sync_info=mybir.SyncInfo(on_wait=[w], on_update=[]),
                            )
                        )
                    ins.sync_info = mybir.SyncInfo(
                        on_wait=waits[-1:], on_update=list(si.on_update or [])
                    )
                    changed = True
                out.append(ins)
            if changed:
                bb.instructions[:] = out


def _build():
    import concourse.bass as bass
    import concourse.mybir as mybir
    import concourse.tile as tile

    f32 = mybir.dt.float32
    f32r = mybir.dt.float32r
    Exp = mybir.ActivationFunctionType.Exp

    nc = bass.Bass(target_bir_lowering=False)

    xt = nc.dram_tensor("xt", [BPC, D, N], f32, kind="ExternalInput")
    a_mat = nc.dram_tensor("a_mat", [D, D], f32, kind="ExternalInput")
    wvt = nc.dram_tensor("wvt", [D, D], f32, kind="ExternalInput")
    ones_col_d = nc.dram_tensor("ones_col", [128, 1], f32, kind="ExternalInput")
    ones_row_d = nc.dram_tensor("ones_row", [1, 128], f32, kind="ExternalInput")
    out_t = nc.dram_tensor("out_t", [BPC, D, N], f32, kind="ExternalOutput")

    with tile.TileContext(nc) as tc:
        with (
            tc.tile_pool(name="wpool", bufs=1) as wpool,
            tc.tile_pool(name="xpool", bufs=2) as xpool,
            tc.tile_pool(name="gpool", bufs=2) as gpool,
            tc.tile_pool(name="vpool", bufs=2) as vpool,
            tc.tile_pool(name="epool", bufs=3) as epool,
            tc.tile_pool(name="eapool", bufs=2) as eapool,
            tc.tile_pool(name="opool", bufs=2) as opool,
            tc.tile_pool(name="rpool", bufs=2) as rpool,
            tc.tile_pool(name="dpool", bufs=2) as dpool,
            tc.tile_pool(name="ps_ws", bufs=3, space="PSUM") as ps_ws,
            tc.tile_pool(name="ps_ot", bufs=4, space="PSUM") as ps_ot,
            tc.tile_pool(name="ps_den", bufs=1, space="PSUM") as ps_den,
        ):
            # --- weights / constants (once) ---
            a_sb = []
            for c in range(DC):
                t = wpool.tile([128, D], f32, tag=f"a{c}", name=f"a_sb{c}")
                nc.sync.dma_start(
                    t[:].bitcast(f32r),
                    a_mat[c * 128:(c + 1) * 128, :].bitcast(f32r),
                )
                a_sb.append(t)
            ones_col = wpool.tile([128, 1], f32, tag="onec")
            nc.sync.dma_start(ones_col[:].bitcast(f32r), ones_col_d[:].bitcast(f32r))
            ones_row = wpool.tile([1, 128], f32, tag="oner")
            nc.sync.dma_start(ones_row[:].bitcast(f32r), ones_row_d[:].bitcast(f32r))
            wvt_sb = []
            for c in range(DC):
                t = wpool.tile([128, D], f32, tag=f"wvt{c}", name=f"wvt_sb{c}")
                nc.sync.dma_start(
                    t[:].bitcast(f32r),
                    wvt[c * 128:(c + 1) * 128, :].bitcast(f32r),
                )
                wvt_sb.append(t)

            def load_xt(b):
                xts = []
                for c in range(DC):
                    t = xpool.tile([128, N], f32, tag=f"xt{c}", name=f"xt_b{b}c{c}")
                    nc.scalar.dma_start(
                        t[:].bitcast(f32r),
                        xt[b, c * 128:(c + 1) * 128, :].bitcast(f32r),
                    )
                    xts.append(t)
                return xts

            def gt_phase(b, xts):
                gt_sb = gpool.tile([128, DC * N], f32, tag="gt", name=f"gt_b{b}")
                for m in range(DC):
                    for h in range(QH):
                        pg = ps_ws.tile([128, 512], f32, tag="ws", name=f"pg{b}{m}{h}")
                        for k4 in range(DC):
                            nc.tensor.matmul(
                                pg[:],
                                a_sb[k4][:, m * 128:(m + 1) * 128].bitcast(f32r),
                                xts[k4][:, h * 512:(h + 1) * 512].bitcast(f32r),
                                start=(k4 == 0), stop=(k4 == DC - 1),
                            )
                        nc.scalar.copy(
                            gt_sb[:, m * N + h * 512:m * N + (h + 1) * 512].bitcast(f32r),
                            pg[:],
                        )
                return gt_sb

            def v_phase(b, xts):
                v_sb = vpool.tile([128, KC * D], f32, tag="v", name=f"v_b{b}")
                for m in range(KC):
                    pv = ps_ws.tile([128, 512], f32, tag="ws", name=f"pv{b}{m}")
                    for k4 in range(DC):
                        nc.tensor.matmul(
                            pv[:],
                            xts[k4][:, m * 128:(m + 1) * 128].bitcast(f32r),
                            wvt_sb[k4][:].bitcast(f32r),
                            start=(k4 == 0), stop=(k4 == DC - 1),
                        )
                    nc.scalar.copy(
                        v_sb[:, m * D:(m + 1) * D].bitcast(f32r), pv[:]
                    )
                return v_sb

            def gt_phase_k4outer(b, xts):
                gt_sb = gpool.tile([128, DC * N], f32, tag="gt", name=f"gt_b{b}")
                grp = {}
                for i, (m, h) in enumerate([(m, h) for m in range(DC) for h in range(QH)]):
                    pool = [ps_ws, ps_ws, ps_ws, ps_ot, ps_ot, ps_ot, ps_ot, ps_den][i]
                    grp[(m, h)] = pool.tile([128, 512], f32, tag=["ws", "ws", "ws", "ot", "ot", "ot", "ot", "den"][i], name=f"pg0_{m}{h}")
                for k4 in range(DC):
                    for m in range(DC):
                        for h in range(QH):
                            nc.tensor.matmul(
                                grp[(m, h)][:],
                                a_sb[k4][:, m * 128:(m + 1) * 128].bitcast(f32r),
                                xts[k4][:, h * 512:(h + 1) * 512].bitcast(f32r),
                                start=(k4 == 0), stop=(k4 == DC - 1),
                            )
                for m in range(DC):
                    for h in range(QH):
                        nc.scalar.copy(
                            gt_sb[:, m * N + h * 512:m * N + (h + 1) * 512].bitcast(f32r),
                            grp[(m, h)][:],
                        )
                return gt_sb

            def v_phase_k4outer(b, xts):
                v_sb = vpool.tile([128, KC * D], f32, tag="v", name=f"v_b{b}")
                grp = {}
                for m in range(KC):
                    pool = [ps_ws, ps_ws, ps_ws, ps_ot, ps_ot, ps_ot, ps_ot, ps_den][m]
                    grp[m] = pool.tile([128, 512], f32, tag=["ws", "ws", "ws", "ot", "ot", "ot", "ot", "den"][m], name=f"pv0_{m}")
                for k4 in range(DC):
                    for m in range(KC):
                        nc.tensor.matmul(
                            grp[m][:],
                            xts[k4][:, m * 128:(m + 1) * 128].bitcast(f32r),
                            wvt_sb[k4][:].bitcast(f32r),
                            start=(k4 == 0), stop=(k4 == DC - 1),
                        )
                for m in range(KC):
                    nc.scalar.copy(
                        v_sb[:, m * D:(m + 1) * D].bitcast(f32r), grp[m][:]
                    )
                return v_sb

            def ph2_compute(b, h, xts, gt_sb, v_sb):
                p_den = ps_den.tile([1, 512], f32, tag="den", name=f"den{b}{h}")
                p_ot = [ps_ot.tile([128, 512], f32, tag="ot", name=f"p_ot{b}{h}{m}")
                        for m in range(DC)]
                # E running sum on VectorE (replaces 7 of 8 ones-matmuls on PE)
                ea = [eapool.tile([128, 512], f32, tag="ea0", name=f"ea0_{b}{h}"),
                      eapool.tile([128, 512], f32, tag="ea1", name=f"ea1_{b}{h}")]
                for kc in range(KC):
                    p_st = ps_ws.tile([128, 512], f32, tag="ws", name=f"st{b}{h}{kc}")
                    for k4 in range(DC):
                        nc.tensor.matmul(
                            p_st[:],
                            xts[k4][:, kc * 128:(kc + 1) * 128].bitcast(f32r),
                            gt_sb[:, k4 * N + h * 512:k4 * N + (h + 1) * 512].bitcast(f32r),
                            start=(k4 == 0), stop=(k4 == DC - 1),
                        )
                    e_sb = epool.tile([128, 512], f32, tag="e", name=f"e{b}{h}{kc}")
                    nc.scalar.activation(e_sb[:].bitcast(f32r), p_st[:], Exp)
                    if kc == 0:
                        nc.vector.tensor_copy(ea[0][:], e_sb[:])
                    else:
                        nc.vector.tensor_add(
                            ea[kc % 2][:], ea[(kc + 1) % 2][:], e_sb[:]
                        )
                    for m in range(DC):
                        nc.tensor.matmul(
                            p_ot[m][:],
                            v_sb[:, kc * D + m * 128:kc * D + (m + 1) * 128].bitcast(f32r),
                            e_sb[:].bitcast(f32r),
                            start=(kc == 0), stop=(kc == KC - 1),
                        )
                ea_r = eapool.tile([128, 512], f32, tag="ear", name=f"ear{b}{h}")
                nc.vector.tensor_copy(ea_r[:].bitcast(f32r), ea[(KC - 1) % 2][:])
                nc.tensor.matmul(
                    p_den[:], ones_col[:].bitcast(f32r), ea_r[:].bitcast(f32r),
                    start=True, stop=True,
                )
                return p_den, p_ot

            def ph2_evict(b, h, p_den, p_ot):
                den_sb = dpool.tile([1, 512], f32, tag="densb", name=f"dsb{b}{h}")
                nc.vector.tensor_copy(den_sb[:].bitcast(f32r), p_den[:])
                otraw = opool.tile([128, DC * 512], f32, tag="otraw", name=f"orw{b}{h}")
                for m in range(DC):
                    nc.scalar.copy(otraw[:, m * 512:(m + 1) * 512], p_ot[m][:])
                return den_sb, otraw

            def ph2_norm(b, h, den_sb, otraw):
                p_bc = ps_ws.tile([128, 512], f32, tag="ws", name=f"bc{b}{h}")
                nc.tensor.matmul(
                    p_bc[:], ones_row[:].bitcast(f32r), den_sb[:].bitcast(f32r)
                )
                ln_sb = rpool.tile([128, 512], f32, tag="ln", name=f"ln{b}{h}")
                nc.scalar.activation(ln_sb[:], p_bc[:], mybir.ActivationFunctionType.Ln)
                rc_sb = rpool.tile([128, 512], f32, tag="rc", name=f"rc{b}{h}")
                nc.scalar.activation(rc_sb[:], ln_sb[:], mybir.ActivationFunctionType.Exp,
                                     scale=-1.0)
                ot_sb = opool.tile([128, DC * 512], f32, tag="ot", name=f"osb{b}{h}")
                for g in range(2):
                    for m in (2 * g, 2 * g + 1):
                        nc.vector.tensor_mul(
                            ot_sb[:, m * 512:(m + 1) * 512],
                            otraw[:, m * 512:(m + 1) * 512], rc_sb[:]
                        )
                    (nc.scalar if g == 0 else nc.sync).dma_start(
                        out_t[b, g * 256:(g + 1) * 256, h * 512:(h + 1) * 512].rearrange(
                            "(m p) q -> p m q", p=128
                        ),
                        ot_sb[:, g * 1024:(g + 1) * 1024].rearrange(
                            "p (m q) -> p m q", m=2
                        ),
                    )

            # software pipeline: phase-1 of batch b+1 fills the PE boundary
            # stalls of batch b's phase-2 (PE executes in program order).
            xts = load_xt(0)
            gt_sb = gt_phase_k4outer(0, xts)
            v_sb = v_phase_k4outer(0, xts)
            state = (xts, gt_sb, v_sb)
            for b in range(BPC):
                xts, gt_sb, v_sb = state
                p_den, p_ot = ph2_compute(b, 0, xts, gt_sb, v_sb)
                den_sb, otraw = ph2_evict(b, 0, p_den, p_ot)
                if b + 1 < BPC:
                    nxts = load_xt(b + 1)
                    ngt = gt_phase(b + 1, nxts)
                ph2_norm(b, 0, den_sb, otraw)
                p_den, p_ot = ph2_compute(b, 1, xts, gt_sb, v_sb)
                den_sb, otraw = ph2_evict(b, 1, p_den, p_ot)
                if b + 1 < BPC:
                    nv = v_phase(b + 1, nxts)
                    state = (nxts, ngt, nv)
                ph2_norm(b, 1, den_sb, otraw)
    return nc


def _prepare_inputs(embeddings, Wq, Wk, Wv):
    xt_all = np.ascontiguousarray(embeddings.transpose(0, 2, 1)).astype(
        np.float32, copy=False
    )
    a_mat = (
        Wq.astype(np.float64).T @ Wk.astype(np.float64) / np.sqrt(float(D))
    ).astype(np.float32)
    wvt = np.ascontiguousarray(Wv.T).astype(np.float32, copy=False)
    ones_col = np.ones((128, 1), np.float32)
    ones_row = np.ones((1, 128), np.float32)
    in_maps = []
    for i in range(NCORES):
        in_maps.append(
            {
                "xt": np.ascontiguousarray(xt_all[i * BPC:(i + 1) * BPC]),
                "a_mat": a_mat,
                "wvt": wvt,
                "ones_col": ones_col,
                "ones_row": ones_row,
            }
        )
    return in_maps


def _get_nc():
    if "nc" not in _cache:
        nc = _build()
        _split_sync_waits(nc)
        _cache["nc"] = nc
    return _cache["nc"]


def _assemble(results):
    out = np.empty((B, N, D), np.float32)
    for i in range(NCORES):
        ot = results[i]["out_t"]  # [BPC, D, N]
        out[i * BPC:(i + 1) * BPC] = ot.transpose(0, 2, 1)
    return out


def kernel(embeddings, Wq, Wk, Wv):
    from concourse.bass_utils import run_bass_kernel_spmd

    embeddings = np.asarray(embeddings, dtype=np.float32)
    in_maps = _prepare_inputs(
        embeddings, np.asarray(Wq), np.asarray(Wk), np.asarray(Wv)
    )
    res = run_bass_kernel_spmd(_get_nc(), in_maps, list(range(NCORES)))
    return _assemble(res.results)



# revision 6
# speedup vs baseline: 1.0208x; 1.0208x over previous
"""Trainium2 Bass kernel for nn_Attention: single-head attention,
B=32, N=1024, DIM=512, fp32.

    q = X @ Wq.T ; k = X @ Wk.T ; v = X @ Wv.T
    out = softmax((q k^T)/sqrt(D)) @ v

Strategy (8 NeuronCores, data-parallel over batch, 4 batches/core):
  - Host folds A = (Wq.T @ Wk)/sqrt(D)  so scores = X A X.T  — saves one
    projection-sized matmul per batch and needs only X (transposed) on
    device.
  - All tensors live transposed on device: XT [d, n], GT = (X A).T,
    V [n, e], scores ST [k, q] (k on partitions).  Softmax runs along
    the partition axis: exp on ScalarE, partition sums via a ones-vector
    matmul, broadcast of 1/denom via a rank-1 ones matmul, normalization
    on VectorE.  Attention output is produced transposed (OT [e, q]) and
    the host transposes it back.
  - Matmuls use the float32r PE mode (full-rate fp32 streaming).
  - Flat software pipeline over h-tiles s = 2b+h (512 q-cols each):
    slot s interleaves S(s) matmuls with O(s-1) matmuls and the gt/v
    projection matmuls of batch b+1; evict/normalize/store of tile s-2
    ride along on Scalar/Vector/DMA.  PE bubbles also reset the PE
    p-state (~3us of half-clock each), so the stream must stay dense.
    The last tile's O phase runs as two 256-wide halves so its norm
    chain overlaps the second half.
"""
import numpy as np

B, N, D = 32, 1024, 512
NCORES = 8
BPC = B // NCORES          # batches per core
DC = D // 128              # 4 chunks of 128 along d / e
KC = N // 128              # 8 chunks of 128 along k
NSLOT = 2 * BPC            # h-tiles per core (512 q-cols each)

_cache = {}


def _split_sync_waits(nc):
    """walrus on this image accepts at most ONE semaphore wait per
    instruction; hoist extras onto InstNoOp carriers on the same engine
    (same-engine program order preserves the gating)."""
    import concourse.mybir as mybir

    ctr = 0
    for f in nc.m.functions:
        for bb in f.blocks:
            out = []
            changed = False
            for ins in bb.instructions:
                si = getattr(ins, "sync_info", None)
                waits = list(si.on_wait) if si and si.on_wait else []
                if len(waits) > 1:
                    for w in waits[:-1]:
                        ctr += 1
                        out.append(
                            mybir.InstNoOp(
                                name=f"wsplit-{ctr}",
                                engine=ins.engine,
                                bass_nofuse=True,
                                sync_info=mybir.SyncInfo(on_wait=[w], on_update=[]),
                            )
                        )
                    ins.sync_info = mybir.SyncInfo(
                        on_wait=waits[-1:], on_update=list(si.on_update or [])
                    )
                    changed = True
                out.append(ins)
            if changed:
                bb.instructions[:] = out


def _build():
    import concourse.bass as bass
    import concourse.mybir as mybir
    import concourse.tile as tile

    f32 = mybir.dt.float32
    f32r = mybir.dt.float32r
    Exp = mybir.ActivationFunctionType.Exp
    Ln = mybir.ActivationFunctionType.Ln

    nc = bass.Bass(target_bir_lowering=False)

    xt = nc.dram_tensor("xt", [BPC, D, N], f32, kind="ExternalInput")
    a_mat = nc.dram_tensor("a_mat", [D, D], f32, kind="ExternalInput")
    wvt = nc.dram_tensor("wvt", [D, D], f32, kind="ExternalInput")
    ones_col_d = nc.dram_tensor("ones_col", [128, 1], f32, kind="ExternalInput")
    ones_row_d = nc.dram_tensor("ones_row", [1, 128], f32, kind="ExternalInput")
    out_t = nc.dram_tensor("out_t", [BPC, D, N], f32, kind="ExternalOutput")

    with tile.TileContext(nc) as tc:
        with (
            tc.tile_pool(name="wpool", bufs=1) as wpool,
            tc.tile_pool(name="xpool", bufs=3) as xpool,
            tc.tile_pool(name="gpool", bufs=2) as gpool,
            tc.tile_pool(name="vpool", bufs=2) as vpool,
            tc.tile_pool(name="epool", bufs=2) as epool,
            tc.tile_pool(name="eapool", bufs=2) as eapool,
            tc.tile_pool(name="opool", bufs=2) as opool,
            tc.tile_pool(name="rpool", bufs=2) as rpool,
            tc.tile_pool(name="dpool", bufs=2) as dpool,
            tc.tile_pool(name="ps_ws", bufs=3, space="PSUM") as ps_ws,
            tc.tile_pool(name="ps_ot", bufs=1, space="PSUM") as ps_ot,
            tc.tile_pool(name="ps_db", bufs=1, space="PSUM") as ps_db,
        ):
            # ---------------- weights / constants (sync queue) ----------
            a_sb = []
            for c in range(DC):
                t = wpool.tile([128, D], f32, tag=f"a{c}", name=f"a_sb{c}")
                nc.sync.dma_start(t[:].bitcast(f32r), a_mat[c * 128:(c + 1) * 128, :].bitcast(f32r))
                a_sb.append(t)
            ones_col = wpool.tile([128, 1], f32, tag="onec")
            nc.sync.dma_start(ones_col[:].bitcast(f32r), ones_col_d[:].bitcast(f32r))
            ones_row = wpool.tile([1, 128], f32, tag="oner")
            nc.sync.dma_start(ones_row[:].bitcast(f32r), ones_row_d[:].bitcast(f32r))
            wvt_sb = []
            for c in range(DC):
                t = wpool.tile([128, D], f32, tag=f"wvt{c}", name=f"wvt_sb{c}")
                nc.sync.dma_start(t[:].bitcast(f32r), wvt[c * 128:(c + 1) * 128, :].bitcast(f32r))
                wvt_sb.append(t)

            # ---------------- xt half-tiles (scalar queue) --------------
            # xh[b][k4][hh] = X^T[d-chunk k4, n-half hh]  [128, 512]
            xh = {}

            def load_xt(b):
                tiles = []
                for k4 in range(DC):
                    tiles.append(
                        [
                            xpool.tile(
                                [128, 512], f32, tag=f"x{k4}{hh}",
                                name=f"x_b{b}_{k4}{hh}",
                            )
                            for hh in range(2)
                        ]
                    )
                # hh-outer issue order so the hh=0 set lands first
                for hh in range(2):
                    for k4 in range(DC):
                        nc.scalar.dma_start(
                            tiles[k4][hh][:].bitcast(f32r),
                            xt[
                                b, k4 * 128:(k4 + 1) * 128,
                                hh * 512:(hh + 1) * 512,
                            ].bitcast(f32r),
                        )
                xh[b] = tiles

            # xt stationary slice (column chunk kc of N)
            def xslice(b, k4, kc):
                return xh[b][k4][kc // 4][:, (kc % 4) * 128:((kc % 4) + 1) * 128]

            # ---------------- per-batch / per-slot state ----------------
            gt_sb = {}   # [128, DC*N]  G^T chunks at cols m*N + hh*512
            v_sb = {}    # [128, KC*D]  V chunks at cols kc*D
            e_sb = {}    # e_sb[s][kc]  exp tiles [128, 512]
            ea_fin = {}  # final running-sum tile per slot
            den_sb = {}  # [1, 512]
            rc_sb = {}   # [128, 512] 1/denom broadcast
            otraw = {}   # [128, DC*512] raw O^T per slot
            p_ot = {}    # p_ot[s] = 4 psum accumulators

            # ---------------- emitters ----------------------------------
            def gt_group(bn, m, hh):
                grp = ps_ws.tile([128, 512], f32, tag="ws", name=f"pg{bn}{m}{hh}")
                for k4 in range(DC):
                    nc.tensor.matmul(
                        grp[:],
                        a_sb[k4][:, m * 128:(m + 1) * 128].bitcast(f32r),
                        xh[bn][k4][hh][:].bitcast(f32r),
                        start=(k4 == 0), stop=(k4 == DC - 1),
                    )
                nc.scalar.copy(
                    gt_sb[bn][
                        :, m * N + hh * 512:m * N + (hh + 1) * 512
                    ].bitcast(f32r),
                    grp[:],
                )

            def v_group(bn, kc):
                grp = ps_ws.tile([128, 512], f32, tag="ws", name=f"pv{bn}{kc}")
                for k4 in range(DC):
                    nc.tensor.matmul(
                        grp[:],
                        xslice(bn, k4, kc).bitcast(f32r),
                        wvt_sb[k4][:].bitcast(f32r),
                        start=(k4 == 0), stop=(k4 == DC - 1),
                    )
                nc.scalar.copy(v_sb[bn][:, kc * D:(kc + 1) * D].bitcast(f32r), grp[:])

            def s_group(s, kc):
                b, h = s // 2, s % 2
                p_st = ps_ws.tile([128, 512], f32, tag="ws", name=f"st{s}_{kc}")
                for k4 in range(DC):
                    nc.tensor.matmul(
                        p_st[:],
                        xslice(b, k4, kc).bitcast(f32r),
                        gt_sb[b][
                            :, k4 * N + h * 512:k4 * N + (h + 1) * 512
                        ].bitcast(f32r),
                        start=(k4 == 0), stop=(k4 == DC - 1),
                    )
                e = epool.tile([128, 512], f32, tag=f"e{kc}", name=f"e{s}_{kc}")
                nc.scalar.activation(e[:].bitcast(f32r), p_st[:], Exp)
                e_sb[s][kc] = e
                ea = eapool.tile(
                    [128, 512], f32, tag=f"ea{kc % 2}", name=f"ea{s}_{kc}"
                )
                if kc == 0:
                    nc.vector.tensor_copy(ea[:].bitcast(f32r), e[:])
                else:
                    nc.vector.tensor_add(ea[:].bitcast(f32r), ea_fin[s][:], e[:])
                ea_fin[s] = ea

            def o_group(t, kc, lo=0, hi=512, tiles=None):
                """4 O^T matmuls (m inner) accumulating tile t, step kc,
                e-columns lo:hi."""
                bt = t // 2
                tiles = tiles if tiles is not None else p_ot[t]
                for m in range(DC):
                    nc.tensor.matmul(
                        tiles[m][:, 0:hi - lo],
                        v_sb[bt][
                            :, kc * D + m * 128:kc * D + (m + 1) * 128
                        ].bitcast(f32r),
                        e_sb[t][kc][:, lo:hi].bitcast(f32r),
                        start=(kc == 0), stop=(kc == KC - 1),
                    )

            def den_mm(t):
                p = ps_db.tile([1, 512], f32, tag="db", name=f"den{t}")
                nc.tensor.matmul(
                    p[:], ones_col[:].bitcast(f32r), ea_fin[t][:].bitcast(f32r),
                    start=True, stop=True,
                )
                d = dpool.tile([1, 512], f32, tag="den", name=f"dsb{t}")
                nc.scalar.copy(d[:].bitcast(f32r), p[:])
                den_sb[t] = d

            def bc_chain(t, lo=0, hi=512, part=""):
                """broadcast 1/denom over cols lo:hi of tile t."""
                w = hi - lo
                p = ps_db.tile([128, 512], f32, tag="db", name=f"pbc{t}{part}")
                nc.tensor.matmul(
                    p[:, 0:w],
                    ones_row[:].bitcast(f32r),
                    den_sb[t][:, lo:hi].bitcast(f32r),
                    start=True, stop=True,
                )
                ln = rpool.tile([128, 512], f32, tag="ln", name=f"ln{t}{part}")
                nc.scalar.activation(ln[:, 0:w], p[:, 0:w], Ln)
                if t not in rc_sb:
                    rc_sb[t] = rpool.tile([128, 512], f32, tag="rc", name=f"rc{t}")
                nc.scalar.activation(rc_sb[t][:, lo:hi], ln[:, 0:w], Exp, scale=-1.0)

            def evict_ot(t, m, eng, lo=0, hi=512, tiles=None):
                w = hi - lo
                tiles = tiles if tiles is not None else p_ot[t]
                dst = otraw[t][:, m * 512 + lo:m * 512 + hi]
                if eng == "s":
                    nc.scalar.copy(dst, tiles[m][:, 0:w])
                else:
                    nc.vector.tensor_copy(dst, tiles[m][:, 0:w])

            def mul_ot(t, m, lo=0, hi=512):
                sl = otraw[t][:, m * 512 + lo:m * 512 + hi]
                nc.vector.tensor_mul(sl, sl, rc_sb[t][:, lo:hi])

            def dma_out(t, g, eng, lo=0, hi=512):
                """store one 256-row g-half of tile t (q-cols lo:hi)."""
                b, h = t // 2, t % 2
                dst = out_t[
                    b, g * 256:(g + 1) * 256, h * 512 + lo:h * 512 + hi
                ].rearrange("(m p) q -> p m q", p=128)
                q = nc.scalar if eng == "s" else nc.sync
                for j in range(2):
                    m = 2 * g + j
                    q.dma_start(
                        dst[:, j:j + 1, :],
                        otraw[t][:, m * 512 + lo:m * 512 + hi].rearrange(
                            "p (m q) -> p m q", m=1
                        ),
                    )

            def alloc_slot(s):
                e_sb[s] = {}
                otraw[s] = opool.tile(
                    [128, DC * 512], f32, tag="otraw", name=f"orw{s}"
                )
                p_ot[s] = [
                    ps_ot.tile([128, 512], f32, tag=f"ot{m}", name=f"pot{s}{m}")
                    for m in range(DC)
                ]

            # ================= emission ================================
            load_xt(0)
            load_xt(1)

            for b in range(BPC):
                gt_sb[b] = gpool.tile([128, DC * N], f32, tag="gt", name=f"gt{b}")
                v_sb[b] = vpool.tile([128, KC * D], f32, tag="v", name=f"v{b}")

            # ---- prologue: gt(0), k4-outer across 8 psum groups
            # (3x ws + 1x db + 4x ot); (m, hh=0) groups first so S(0,h0)
            # can start as soon as they are evicted.
            grp_order = [(m, 0) for m in range(DC)] + [(m, 1) for m in range(DC)]
            grp = {}
            for i, (m, hh) in enumerate(grp_order):
                if i < 3:
                    grp[(m, hh)] = ps_ws.tile(
                        [128, 512], f32, tag="ws", name=f"pg0_{m}{hh}"
                    )
                elif i == 3:
                    grp[(m, hh)] = ps_db.tile(
                        [128, 512], f32, tag="db", name=f"pg0_{m}{hh}"
                    )
                else:
                    grp[(m, hh)] = ps_ot.tile(
                        [128, 512], f32, tag=f"ot{i - 4}", name=f"pg0_{m}{hh}"
                    )
            # consume xt chunks in DMA-arrival order: (k4, hh) hh-outer
            for hh in range(2):
                for k4 in range(DC):
                    for m in range(DC):
                        nc.tensor.matmul(
                            grp[(m, hh)][:],
                            a_sb[k4][:, m * 128:(m + 1) * 128].bitcast(f32r),
                            xh[0][k4][hh][:].bitcast(f32r),
                            start=(k4 == 0), stop=(k4 == DC - 1),
                        )
                for m in range(DC):
                    nc.scalar.copy(
                        gt_sb[0][
                            :, m * N + hh * 512:m * N + (hh + 1) * 512
                        ].bitcast(f32r),
                        grp[(m, hh)][:],
                    )

            # ---- steady slots -----------------------------------------
            for s in range(NSLOT):
                b, h = s // 2, s % 2
                alloc_slot(s)
                if h == 1 and b + 2 < BPC:
                    load_xt(b + 2)

                for kc in range(KC):
                    if kc == 0 and s >= 2:
                        # free the ot psum banks first thing on Scalar/
                        # Vector so this slot's O groups are not blocked
                        evict_ot(s - 2, 0, "s")
                        evict_ot(s - 2, 1, "v")
                        evict_ot(s - 2, 2, "s")
                        evict_ot(s - 2, 3, "v")

                    s_group(s, kc)

                    if kc == 0 and s >= 1:
                        den_mm(s - 1)
                    if kc == 2 and s >= 1:
                        bc_chain(s - 1)
                    if kc == 3 and s >= 2:
                        for m in range(DC):
                            mul_ot(s - 2, m)
                    if kc == 4 and s >= 2:
                        dma_out(s - 2, 0, "s")
                    if kc == 5 and s >= 2:
                        dma_out(s - 2, 1, "y")

                    if s >= 1:
                        o_group(s - 1, kc)
                    else:
                        v_group(0, kc)  # v(0) occupies slot 0's O position

                    if h == 0 and b + 1 < BPC:
                        gt_group(b + 1, kc % 4, kc // 4)
                    elif h == 1 and b + 1 < BPC:
                        v_group(b + 1, kc)

            # ---- epilogue: tile L's O phase in two 256-col halves -----
            L = NSLOT - 1
            # free ot banks (tile L-2 was evicted in slot L; L-1 now)
            evict_ot(L - 1, 0, "s")
            evict_ot(L - 1, 1, "v")
            evict_ot(L - 1, 2, "s")
            evict_ot(L - 1, 3, "v")
            den_mm(L)
            for kc in range(KC):
                o_group(L, kc, 0, 256)
                if kc == 1:
                    bc_chain(L, 0, 256, part="a")
                if kc == 2:
                    for m in range(DC):
                        mul_ot(L - 1, m)
                if kc == 3:
                    dma_out(L - 1, 0, "s")
                if kc == 4:
                    dma_out(L - 1, 1, "y")
            # half-a evictions free banks chunk by chunk for half-b
            for m in range(DC):
                evict_ot(L, m, "s" if m % 2 == 0 else "v", 0, 256)
            ot_b = [
                ps_ot.tile([128, 512], f32, tag=f"ot{m}", name=f"potb{m}")
                for m in range(DC)
            ]
            for kc in range(KC):
                o_group(L, kc, 256, 512, tiles=ot_b)
                if kc == 0:
                    bc_chain(L, 256, 512, part="b")
                if kc == 1:
                    for m in range(DC):
                        mul_ot(L, m, 0, 256)
                if kc == 3:
                    dma_out(L, 0, "s", 0, 256)
                if kc == 4:
                    dma_out(L, 1, "y", 0, 256)
            for m in range(DC):
                evict_ot(L, m, "s" if m % 2 == 0 else "v", 256, 512, tiles=ot_b)
            for m in range(DC):
                mul_ot(L, m, 256, 512)
            dma_out(L, 0, "s", 256, 512)
            dma_out(L, 1, "y", 256, 512)
    return nc


def _prepare_inputs(embeddings, Wq, Wk, Wv):
    xt_all = np.ascontiguousarray(embeddings.transpose(0, 2, 1)).astype(
        np.float32, copy=False
    )
    a_mat = (
        Wq.astype(np.float64).T @ Wk.astype(np.float64) / np.sqrt(float(D))
    ).astype(np.float32)
    wvt = np.ascontiguousarray(Wv.T).astype(np.float32, copy=False)
    ones_col = np.ones((128, 1), np.float32)
    ones_row = np.ones((1, 128), np.float32)
    in_maps = []
    for i in range(NCORES):
        in_maps.append(
            {
                "xt": np.ascontiguousarray(xt_all[i * BPC:(i + 1) * BPC]),
                "a_mat": a_mat,
                "wvt": wvt,
                "ones_col": ones_col,
                "ones_row": ones_row,
            }
        )
    return in_maps


def _get_nc():
    if "nc" not in _cache:
        nc = _build()
        _split_sync_waits(nc)
        _cache["nc"] = nc
    return _cache["nc"]


def _assemble(results):
    out = np.empty((B, N, D), np.float32)
    for i in range(NCORES):
        ot = results[i]["out_t"]  # [BPC, D, N]
        out[i * BPC:(i + 1) * BPC] = ot.transpose(0, 2, 1)
    return out


def kernel(embeddings, Wq, Wk, Wv):
    from concourse.bass_utils import run_bass_kernel_spmd

    embeddings = np.asarray(embeddings, dtype=np.float32)
    in_maps = _prepare_inputs(
        embeddings, np.asarray(Wq), np.asarray(Wk), np.asarray(Wv)
    )
    res = run_bass_kernel_spmd(_get_nc(), in_maps, list(range(NCORES)))
    return _assemble(res.results)


# revision 7
# speedup vs baseline: 1.0501x; 1.0287x over previous
"""Trainium2 Bass kernel for nn_Attention: single-head attention,
B=32, N=1024, DIM=512, fp32.

    q = X @ Wq.T ; k = X @ Wk.T ; v = X @ Wv.T
    out = softmax((q k^T)/sqrt(D)) @ v

Strategy (8 NeuronCores, data-parallel over batch, 4 batches/core):
  - Host folds A = (Wq.T @ Wk)/sqrt(D)  so scores = X A X.T  — saves one
    projection-sized matmul per batch and needs only X (transposed) on
    device.
  - All tensors live transposed on device: XT [d, n], GT = (X A).T,
    V [n, e], scores ST [k, q] (k on partitions).  Softmax runs along
    the partition axis: exp on ScalarE, partition sums via a ones-vector
    matmul, broadcast of 1/denom via a rank-1 ones matmul, normalization
    on VectorE.  Attention output is produced transposed (OT [e, q]) and
    the host transposes it back.
  - Matmuls use the float32r PE mode (full-rate fp32 streaming).
  - Flat software pipeline over h-tiles s = 2b+h (512 q-cols each):
    slot s interleaves S(s) matmuls with O(s-1) matmuls and the gt/v
    projection matmuls of batch b+1; evict/normalize/store of tile s-2
    ride along on Scalar/Vector/DMA.  PE bubbles also reset the PE
    p-state (~3us of half-clock each), so the stream must stay dense.
    The last tile's O phase runs as two 256-wide halves so its norm
    chain overlaps the second half.
"""
import numpy as np

B, N, D = 32, 1024, 512
NCORES = 8
BPC = B // NCORES          # batches per core
DC = D // 128              # 4 chunks of 128 along d / e
KC = N // 128              # 8 chunks of 128 along k
NSLOT = 2 * BPC            # h-tiles per core (512 q-cols each)

_cache = {}


def _split_sync_waits(nc):
    """walrus on this image accepts at most ONE semaphore wait per
    instruction; hoist extras onto InstNoOp carriers on the same engine
    (same-engine program order preserves the gating)."""
    import concourse.mybir as mybir

    ctr = 0
    for f in nc.m.functions:
        for bb in f.blocks:
            out = []
            changed = False
            for ins in bb.instructions:
                si = getattr(ins, "sync_info", None)
                waits = list(si.on_wait) if si and si.on_wait else []
                if len(waits) > 1:
                    for w in waits[:-1]:
                        ctr += 1
                        out.append(
                            mybir.InstNoOp(
                                name=f"wsplit-{ctr}",
                                engine=ins.engine,
                                bass_nofuse=True,
                                sync_info=mybir.SyncInfo(on_wait=[w], on_update=[]),
                            )
                        )
                    ins.sync_info = mybir.SyncInfo(
                        on_wait=waits[-1:], on_update=list(si.on_update or [])
                    )
                    changed = True
                out.append(ins)
            if changed:
                bb.instructions[:] = out


def _build():
    import concourse.bass as bass
    import concourse.mybir as mybir
    import concourse.tile as tile

    f32 = mybir.dt.float32
    f32r = mybir.dt.float32r
    Exp = mybir.ActivationFunctionType.Exp
    Ln = mybir.ActivationFunctionType.Ln

    nc = bass.Bass(target_bir_lowering=False)

    xt = nc.dram_tensor("xt", [BPC, D, N], f32, kind="ExternalInput")
    a_mat = nc.dram_tensor("a_mat", [D, D], f32, kind="ExternalInput")
    wvt = nc.dram_tensor("wvt", [D, D], f32, kind="ExternalInput")
    ones_col_d = nc.dram_tensor("ones_col", [128, 1], f32, kind="ExternalInput")
    ones_row_d = nc.dram_tensor("ones_row", [1, 128], f32, kind="ExternalInput")
    out_t = nc.dram_tensor("out_t", [BPC, D, N], f32, kind="ExternalOutput")

    with tile.TileContext(nc) as tc:
        with (
            tc.tile_pool(name="wpool", bufs=1) as wpool,
            tc.tile_pool(name="xpool", bufs=3) as xpool,
            tc.tile_pool(name="gpool", bufs=2) as gpool,
            tc.tile_pool(name="vpool", bufs=2) as vpool,
            tc.tile_pool(name="epool", bufs=2) as epool,
            tc.tile_pool(name="eapool", bufs=2) as eapool,
            tc.tile_pool(name="opool", bufs=2) as opool,
            tc.tile_pool(name="rpool", bufs=2) as rpool,
            tc.tile_pool(name="dpool", bufs=2) as dpool,
            tc.tile_pool(name="ps_ws", bufs=3, space="PSUM") as ps_ws,
            tc.tile_pool(name="ps_ot", bufs=1, space="PSUM") as ps_ot,
            tc.tile_pool(name="ps_db", bufs=1, space="PSUM") as ps_db,
        ):
            # ---------------- weights / constants (sync queue) ----------
            a_sb = []
            for c in range(DC):
                t = wpool.tile([128, D], f32, tag=f"a{c}", name=f"a_sb{c}")
                nc.sync.dma_start(t[:].bitcast(f32r), a_mat[c * 128:(c + 1) * 128, :].bitcast(f32r))
                a_sb.append(t)
            ones_col = wpool.tile([128, 1], f32, tag="onec")
            nc.sync.dma_start(ones_col[:].bitcast(f32r), ones_col_d[:].bitcast(f32r))
            ones_row = wpool.tile([1, 128], f32, tag="oner")
            nc.sync.dma_start(ones_row[:].bitcast(f32r), ones_row_d[:].bitcast(f32r))
            wvt_sb = []
            for c in range(DC):
                t = wpool.tile([128, D], f32, tag=f"wvt{c}", name=f"wvt_sb{c}")
                nc.sync.dma_start(t[:].bitcast(f32r), wvt[c * 128:(c + 1) * 128, :].bitcast(f32r))
                wvt_sb.append(t)

            # ---------------- xt half-tiles (scalar queue) --------------
            # xh[b][k4][hh] = X^T[d-chunk k4, n-half hh]  [128, 512]
            xh = {}

            def load_xt(b):
                tiles = []
                for k4 in range(DC):
                    tiles.append(
                        [
                            xpool.tile(
                                [128, 512], f32, tag=f"x{k4}{hh}",
                                name=f"x_b{b}_{k4}{hh}",
                            )
                            for hh in range(2)
                        ]
                    )
                # hh-outer issue order so the hh=0 set lands first;
                # for b>=1 the hh=0 set rides the sync queue so both
                # halves stream in parallel
                for hh in range(2):
                    for k4 in range(DC):
                        q = nc.scalar if (b == 0 or hh == 1) else nc.sync
                        q.dma_start(
                            tiles[k4][hh][:].bitcast(f32r),
                            xt[
                                b, k4 * 128:(k4 + 1) * 128,
                                hh * 512:(hh + 1) * 512,
                            ].bitcast(f32r),
                        )
                xh[b] = tiles

            # xt stationary slice (column chunk kc of N)
            def xslice(b, k4, kc):
                return xh[b][k4][kc // 4][:, (kc % 4) * 128:((kc % 4) + 1) * 128]

            # ---------------- per-batch / per-slot state ----------------
            gt_sb = {}   # [128, DC*N]  G^T chunks at cols m*N + hh*512
            v_sb = {}    # [128, KC*D]  V chunks at cols kc*D
            e_sb = {}    # e_sb[s][kc]  exp tiles [128, 512]
            ea_fin = {}  # final running-sum tile per slot
            den_sb = {}  # [1, 512]
            rc_sb = {}   # [128, 512] 1/denom broadcast
            otraw = {}   # [128, DC*512] raw O^T per slot
            p_ot = {}    # p_ot[s] = 4 psum accumulators

            # ---------------- emitters ----------------------------------
            def gt_group(bn, m, hh):
                grp = ps_ws.tile([128, 512], f32, tag="ws", name=f"pg{bn}{m}{hh}")
                for k4 in range(DC):
                    nc.tensor.matmul(
                        grp[:],
                        a_sb[k4][:, m * 128:(m + 1) * 128].bitcast(f32r),
                        xh[bn][k4][hh][:].bitcast(f32r),
                        start=(k4 == 0), stop=(k4 == DC - 1),
                    )
                nc.scalar.copy(
                    gt_sb[bn][
                        :, m * N + hh * 512:m * N + (hh + 1) * 512
                    ].bitcast(f32r),
                    grp[:],
                )

            def v_group(bn, kc):
                grp = ps_ws.tile([128, 512], f32, tag="ws", name=f"pv{bn}{kc}")
                for k4 in range(DC):
                    nc.tensor.matmul(
                        grp[:],
                        xslice(bn, k4, kc).bitcast(f32r),
                        wvt_sb[k4][:].bitcast(f32r),
                        start=(k4 == 0), stop=(k4 == DC - 1),
                    )
                nc.scalar.copy(v_sb[bn][:, kc * D:(kc + 1) * D].bitcast(f32r), grp[:])

            def s_group(s, kc):
                b, h = s // 2, s % 2
                p_st = ps_ws.tile([128, 512], f32, tag="ws", name=f"st{s}_{kc}")
                for k4 in range(DC):
                    nc.tensor.matmul(
                        p_st[:],
                        xslice(b, k4, kc).bitcast(f32r),
                        gt_sb[b][
                            :, k4 * N + h * 512:k4 * N + (h + 1) * 512
                        ].bitcast(f32r),
                        start=(k4 == 0), stop=(k4 == DC - 1),
                    )
                e = epool.tile([128, 512], f32, tag=f"e{kc}", name=f"e{s}_{kc}")
                nc.scalar.activation(e[:].bitcast(f32r), p_st[:], Exp)
                e_sb[s][kc] = e
                ea = eapool.tile(
                    [128, 512], f32, tag=f"ea{kc % 2}", name=f"ea{s}_{kc}"
                )
                if kc == 0:
                    nc.vector.tensor_copy(ea[:].bitcast(f32r), e[:])
                else:
                    nc.vector.tensor_add(ea[:].bitcast(f32r), ea_fin[s][:], e[:])
                ea_fin[s] = ea

            def o_group(t, kc, lo=0, hi=512, tiles=None):
                """4 O^T matmuls (m inner) accumulating tile t, step kc,
                e-columns lo:hi."""
                bt = t // 2
                tiles = tiles if tiles is not None else p_ot[t]
                for m in range(DC):
                    nc.tensor.matmul(
                        tiles[m][:, 0:hi - lo],
                        v_sb[bt][
                            :, kc * D + m * 128:kc * D + (m + 1) * 128
                        ].bitcast(f32r),
                        e_sb[t][kc][:, lo:hi].bitcast(f32r),
                        start=(kc == 0), stop=(kc == KC - 1),
                    )

            def den_mm(t):
                p = ps_db.tile([1, 512], f32, tag="db", name=f"den{t}")
                nc.tensor.matmul(
                    p[:], ones_col[:].bitcast(f32r), ea_fin[t][:].bitcast(f32r),
                    start=True, stop=True,
                )
                d = dpool.tile([1, 512], f32, tag="den", name=f"dsb{t}")
                nc.scalar.copy(d[:].bitcast(f32r), p[:])
                den_sb[t] = d

            def bc_chain(t, lo=0, hi=512, part=""):
                """broadcast 1/denom over cols lo:hi of tile t."""
                w = hi - lo
                p = ps_db.tile([128, 512], f32, tag="db", name=f"pbc{t}{part}")
                nc.tensor.matmul(
                    p[:, 0:w],
                    ones_row[:].bitcast(f32r),
                    den_sb[t][:, lo:hi].bitcast(f32r),
                    start=True, stop=True,
                )
                ln = rpool.tile([128, 512], f32, tag="ln", name=f"ln{t}{part}")
                nc.scalar.activation(ln[:, 0:w], p[:, 0:w], Ln)
                if t not in rc_sb:
                    rc_sb[t] = rpool.tile([128, 512], f32, tag="rc", name=f"rc{t}")
                nc.scalar.activation(rc_sb[t][:, lo:hi], ln[:, 0:w], Exp, scale=-1.0)

            def evict_ot(t, m, eng, lo=0, hi=512, tiles=None):
                w = hi - lo
                tiles = tiles if tiles is not None else p_ot[t]
                dst = otraw[t][:, m * 512 + lo:m * 512 + hi]
                if eng == "s":
                    nc.scalar.copy(dst, tiles[m][:, 0:w])
                else:
                    nc.vector.tensor_copy(dst, tiles[m][:, 0:w])

            def mul_ot(t, m, lo=0, hi=512):
                sl = otraw[t][:, m * 512 + lo:m * 512 + hi]
                nc.vector.tensor_mul(sl, sl, rc_sb[t][:, lo:hi])

            def dma_out(t, g, eng, lo=0, hi=512):
                """store one 256-row g-half of tile t (q-cols lo:hi)."""
                b, h = t // 2, t % 2
                dst = out_t[
                    b, g * 256:(g + 1) * 256, h * 512 + lo:h * 512 + hi
                ].rearrange("(m p) q -> p m q", p=128)
                q = nc.scalar if eng == "s" else nc.sync
                if lo == 0 and hi == 512:
                    q.dma_start(
                        dst,
                        otraw[t][:, 2 * g * 512:(2 * g + 2) * 512].rearrange(
                            "p (m q) -> p m q", m=2
                        ),
                    )
                    return
                for j in range(2):
                    m = 2 * g + j
                    q.dma_start(
                        dst[:, j:j + 1, :],
                        otraw[t][:, m * 512 + lo:m * 512 + hi].rearrange(
                            "p (m q) -> p m q", m=1
                        ),
                    )

            def alloc_slot(s):
                e_sb[s] = {}
                otraw[s] = opool.tile(
                    [128, DC * 512], f32, tag="otraw", name=f"orw{s}"
                )
                p_ot[s] = [
                    ps_ot.tile([128, 512], f32, tag=f"ot{m}", name=f"pot{s}{m}")
                    for m in range(DC)
                ]

            # ================= emission ================================
            load_xt(0)
            load_xt(1)

            for b in range(BPC):
                gt_sb[b] = gpool.tile([128, DC * N], f32, tag="gt", name=f"gt{b}")
                v_sb[b] = vpool.tile([128, KC * D], f32, tag="v", name=f"v{b}")

            # ---- prologue: gt(0), k4-outer across 8 psum groups
            # (3x ws + 1x db + 4x ot); (m, hh=0) groups first so S(0,h0)
            # can start as soon as they are evicted.
            grp_order = [(m, 0) for m in range(DC)] + [(m, 1) for m in range(DC)]
            grp = {}
            for i, (m, hh) in enumerate(grp_order):
                if i < 3:
                    grp[(m, hh)] = ps_ws.tile(
                        [128, 512], f32, tag="ws", name=f"pg0_{m}{hh}"
                    )
                elif i == 3:
                    grp[(m, hh)] = ps_db.tile(
                        [128, 512], f32, tag="db", name=f"pg0_{m}{hh}"
                    )
                else:
                    grp[(m, hh)] = ps_ot.tile(
                        [128, 512], f32, tag=f"ot{i - 4}", name=f"pg0_{m}{hh}"
                    )
            # consume xt chunks in DMA-arrival order: (k4, hh) hh-outer
            for hh in range(2):
                for k4 in range(DC):
                    for m in range(DC):
                        nc.tensor.matmul(
                            grp[(m, hh)][:],
                            a_sb[k4][:, m * 128:(m + 1) * 128].bitcast(f32r),
                            xh[0][k4][hh][:].bitcast(f32r),
                            start=(k4 == 0), stop=(k4 == DC - 1),
                        )
                for m in range(DC):
                    nc.scalar.copy(
                        gt_sb[0][
                            :, m * N + hh * 512:m * N + (hh + 1) * 512
                        ].bitcast(f32r),
                        grp[(m, hh)][:],
                    )

            # ---- steady slots -----------------------------------------
            for s in range(NSLOT):
                b, h = s // 2, s % 2
                alloc_slot(s)
                if h == 1 and b + 2 < BPC:
                    load_xt(b + 2)

                for kc in range(KC):
                    if kc == 0 and s >= 2:
                        # free the ot psum banks first thing on Scalar/
                        # Vector so this slot's O groups are not blocked
                        evict_ot(s - 2, 0, "s")
                        evict_ot(s - 2, 1, "v")
                        evict_ot(s - 2, 2, "s")
                        evict_ot(s - 2, 3, "v")

                    # O leads at the last two steps so its psum stop (and
                    # the next slot's evictions) land earlier
                    if kc >= 6 and s >= 1:
                        o_group(s - 1, kc)

                    s_group(s, kc)

                    if kc == 1 and s >= 1:
                        # kc==1 keeps the den matmul's counter-based waits
                        # clear of the slot-start Scalar/Vector backlog
                        den_mm(s - 1)
                    if kc == 2 and s >= 1:
                        bc_chain(s - 1)
                    if kc == 3 and s >= 2:
                        for m in range(DC):
                            mul_ot(s - 2, m)
                    if kc == 4 and s >= 2:
                        dma_out(s - 2, 0, "s")
                    if kc == 5 and s >= 2:
                        dma_out(s - 2, 1, "y")

                    if s >= 1 and kc < 6:
                        o_group(s - 1, kc)
                    if s == 0:
                        v_group(0, kc)  # v(0) occupies slot 0's O position

                    # gt(1) runs in slot 1 (xt(1) has fully landed by then)
                    if s == 1:
                        gt_group(1, kc % 4, kc // 4)
                        v_group(1, kc)
                    elif h == 0 and b >= 1 and b + 1 < BPC:
                        gt_group(b + 1, kc % 4, kc // 4)
                    elif h == 1 and s > 1 and b + 1 < BPC:
                        v_group(b + 1, kc)

            # ---- epilogue: tile L's O phase in two 256-col halves -----
            L = NSLOT - 1
            # free ot banks (tile L-2 was evicted in slot L; L-1 now)
            evict_ot(L - 1, 0, "s")
            evict_ot(L - 1, 1, "v")
            evict_ot(L - 1, 2, "s")
            evict_ot(L - 1, 3, "v")
            den_mm(L)
            for kc in range(KC):
                o_group(L, kc, 0, 256)
                if kc == 1:
                    bc_chain(L, 0, 256, part="a")
                if kc == 2:
                    for m in range(DC):
                        mul_ot(L - 1, m)
                if kc == 3:
                    dma_out(L - 1, 0, "y")
                if kc == 4:
                    dma_out(L - 1, 1, "y")
            # half-a evictions free banks chunk by chunk for half-b
            for m in range(DC):
                evict_ot(L, m, "s" if m % 2 == 0 else "v", 0, 256)
            ot_b = [
                ps_ot.tile([128, 512], f32, tag=f"ot{m}", name=f"potb{m}")
                for m in range(DC)
            ]
            for kc in range(KC):
                o_group(L, kc, 256, 512, tiles=ot_b)
                if kc == 0:
                    bc_chain(L, 256, 512, part="b")
                if kc == 1:
                    for m in range(DC):
                        mul_ot(L, m, 0, 256)
                if kc == 3:
                    dma_out(L, 0, "s", 0, 256)
                if kc == 4:
                    dma_out(L, 1, "y", 0, 256)
            for m in range(DC):
                evict_ot(L, m, "s" if m % 2 == 0 else "v", 256, 512, tiles=ot_b)
            for m in range(DC):
                mul_ot(L, m, 256, 512)
            dma_out(L, 0, "s", 256, 512)
            dma_out(L, 1, "y", 256, 512)
    return nc


def _prepare_inputs(embeddings, Wq, Wk, Wv):
    xt_all = np.ascontiguousarray(embeddings.transpose(0, 2, 1)).astype(
        np.float32, copy=False
    )
    a_mat = (
        Wq.astype(np.float64).T @ Wk.astype(np.float64) / np.sqrt(float(D))
    ).astype(np.float32)
    wvt = np.ascontiguousarray(Wv.T).astype(np.float32, copy=False)
    ones_col = np.ones((128, 1), np.float32)
    ones_row = np.ones((1, 128), np.float32)
    in_maps = []
    for i in range(NCORES):
        in_maps.append(
            {
                "xt": np.ascontiguousarray(xt_all[i * BPC:(i + 1) * BPC]),
                "a_mat": a_mat,
                "wvt": wvt,
                "ones_col": ones_col,
                "ones_row": ones_row,
            }
        )
    return in_maps


def _get_nc():
    if "nc" not in _cache:
        nc = _build()
        _split_sync_waits(nc)
        _cache["nc"] = nc
    return _cache["nc"]


def _assemble(results):
    out = np.empty((B, N, D), np.float32)
    for i in range(NCORES):
        ot = results[i]["out_t"]  # [BPC, D, N]
        out[i * BPC:(i + 1) * BPC] = ot.transpose(0, 2, 1)
    return out


def kernel(embeddings, Wq, Wk, Wv):
    from concourse.bass_utils import run_bass_kernel_spmd

    embeddings = np.asarray(embeddings, dtype=np.float32)
    in_maps = _prepare_inputs(
        embeddings, np.asarray(Wq), np.asarray(Wk), np.asarray(Wv)
    )
    res = run_bass_kernel_spmd(_get_nc(), in_maps, list(range(NCORES)))
    return _assemble(res.results)


# revision 9
# speedup vs baseline: 1.0718x; 1.0206x over previous
"""Trainium2 Bass kernel for nn_Attention: single-head attention,
B=32, N=1024, DIM=512, fp32.

    q = X @ Wq.T ; k = X @ Wk.T ; v = X @ Wv.T
    out = softmax((q k^T)/sqrt(D)) @ v

Strategy (8 NeuronCores, data-parallel over batch, 4 batches/core):
  - Host folds A = (Wq.T @ Wk)/sqrt(D)  so scores = X A X.T  — saves one
    projection-sized matmul per batch and needs only X (transposed) on
    device.
  - All tensors live transposed on device: XT [d, n], GT = (X A).T,
    V [n, e], scores ST [k, q] (k on partitions).  Softmax runs along
    the partition axis: exp on ScalarE, partition sums via a ones-vector
    matmul, broadcast of 1/denom via a rank-1 ones matmul, normalization
    on VectorE.  Attention output is produced transposed (OT [e, q]) and
    the host transposes it back.
  - Matmuls use the float32r PE mode (full-rate fp32 streaming).
  - Flat software pipeline over h-tiles s = 2b+h (512 q-cols each):
    slot s interleaves S(s) matmuls with O(s-1) matmuls and the gt/v
    projection matmuls of batch b+1; evict/normalize/store of tile s-2
    ride along on Scalar/Vector/DMA.  PE bubbles also reset the PE
    p-state (~3us of half-clock each), so the stream must stay dense.
    The last tile's O phase runs as two 256-wide halves so its norm
    chain overlaps the second half.
"""
import numpy as np

B, N, D = 32, 1024, 512
NCORES = 8
BPC = B // NCORES          # batches per core
DC = D // 128              # 4 chunks of 128 along d / e
KC = N // 128              # 8 chunks of 128 along k
NSLOT = 2 * BPC            # h-tiles per core (512 q-cols each)

_cache = {}


def _split_sync_waits(nc):
    """walrus on this image accepts at most ONE semaphore wait per
    instruction; hoist extras onto InstNoOp carriers on the same engine
    (same-engine program order preserves the gating)."""
    import concourse.mybir as mybir

    ctr = 0
    for f in nc.m.functions:
        for bb in f.blocks:
            out = []
            changed = False
            for ins in bb.instructions:
                si = getattr(ins, "sync_info", None)
                waits = list(si.on_wait) if si and si.on_wait else []
                if len(waits) > 1:
                    for w in waits[:-1]:
                        ctr += 1
                        out.append(
                            mybir.InstNoOp(
                                name=f"wsplit-{ctr}",
                                engine=ins.engine,
                                bass_nofuse=True,
                                sync_info=mybir.SyncInfo(on_wait=[w], on_update=[]),
                            )
                        )
                    ins.sync_info = mybir.SyncInfo(
                        on_wait=waits[-1:], on_update=list(si.on_update or [])
                    )
                    changed = True
                out.append(ins)
            if changed:
                bb.instructions[:] = out


def _build():
    import concourse.bass as bass
    import concourse.mybir as mybir
    import concourse.tile as tile

    f32 = mybir.dt.float32
    f32r = mybir.dt.float32r
    Exp = mybir.ActivationFunctionType.Exp
    Ln = mybir.ActivationFunctionType.Ln

    nc = bass.Bass(target_bir_lowering=False)

    xt = nc.dram_tensor("xt", [BPC, D, N], f32, kind="ExternalInput")
    a_mat = nc.dram_tensor("a_mat", [D, D], f32, kind="ExternalInput")
    wvt = nc.dram_tensor("wvt", [D, D], f32, kind="ExternalInput")
    ones_d = nc.dram_tensor("ones_mat", [128, 128], f32, kind="ExternalInput")
    out_t = nc.dram_tensor("out_t", [BPC, D, N], f32, kind="ExternalOutput")

    with tile.TileContext(nc) as tc:
        with (
            tc.tile_pool(name="wpool", bufs=1) as wpool,
            tc.tile_pool(name="xpool", bufs=3) as xpool,
            tc.tile_pool(name="gpool", bufs=2) as gpool,
            tc.tile_pool(name="vpool", bufs=2) as vpool,
            tc.tile_pool(name="epool", bufs=2) as epool,
            tc.tile_pool(name="eapool", bufs=2) as eapool,
            tc.tile_pool(name="opool", bufs=2) as opool,
            tc.tile_pool(name="rpool", bufs=2) as rpool,
            tc.tile_pool(name="ps_ws", bufs=3, space="PSUM") as ps_ws,
            tc.tile_pool(name="ps_ot", bufs=1, space="PSUM") as ps_ot,
            tc.tile_pool(name="ps_db", bufs=1, space="PSUM") as ps_db,
        ):
            # ---------------- weights / constants (sync queue) ----------
            a_sb = []
            for c in range(DC):
                t = wpool.tile([128, D], f32, tag=f"a{c}", name=f"a_sb{c}")
                nc.sync.dma_start(t[:].bitcast(f32r), a_mat[c * 128:(c + 1) * 128, :].bitcast(f32r))
                a_sb.append(t)
            ones_mat = wpool.tile([128, 128], f32, tag="ones")
            nc.sync.dma_start(ones_mat[:].bitcast(f32r), ones_d[:].bitcast(f32r))
            wvt_sb = []
            for c in range(DC):
                t = wpool.tile([128, D], f32, tag=f"wvt{c}", name=f"wvt_sb{c}")
                nc.sync.dma_start(t[:].bitcast(f32r), wvt[c * 128:(c + 1) * 128, :].bitcast(f32r))
                wvt_sb.append(t)

            # ---------------- xt half-tiles (scalar queue) --------------
            # xh[b][k4][hh] = X^T[d-chunk k4, n-half hh]  [128, 512]
            xh = {}

            def load_xt(b):
                tiles = []
                for k4 in range(DC):
                    tiles.append(
                        [
                            xpool.tile(
                                [128, 512], f32, tag=f"x{k4}{hh}",
                                name=f"x_b{b}_{k4}{hh}",
                            )
                            for hh in range(2)
                        ]
                    )
                # hh-outer issue order so the hh=0 set lands first;
                # for b>=1 the hh=0 set rides the sync queue so both
                # halves stream in parallel
                for hh in range(2):
                    for k4 in range(DC):
                        q = nc.scalar if (b == 0 or hh == 1) else nc.sync
                        q.dma_start(
                            tiles[k4][hh][:].bitcast(f32r),
                            xt[
                                b, k4 * 128:(k4 + 1) * 128,
                                hh * 512:(hh + 1) * 512,
                            ].bitcast(f32r),
                        )
                xh[b] = tiles

            # xt stationary slice (column chunk kc of N)
            def xslice(b, k4, kc):
                return xh[b][k4][kc // 4][:, (kc % 4) * 128:((kc % 4) + 1) * 128]

            # ---------------- per-batch / per-slot state ----------------
            gt_sb = {}   # [128, DC*N]  G^T chunks at cols m*N + hh*512
            v_sb = {}    # [128, KC*D]  V chunks at cols kc*D
            e_sb = {}    # e_sb[s][kc]  exp tiles [128, 512]
            ea_fin = {}  # final running-sum tile per slot
            rc_sb = {}   # [128, 512] 1/denom broadcast
            otraw = {}   # [128, DC*512] raw O^T per slot
            p_ot = {}    # p_ot[s] = 4 psum accumulators

            # ---------------- emitters ----------------------------------
            def gt_group(bn, m, hh):
                grp = ps_ws.tile([128, 512], f32, tag="ws", name=f"pg{bn}{m}{hh}")
                for k4 in range(DC):
                    nc.tensor.matmul(
                        grp[:],
                        a_sb[k4][:, m * 128:(m + 1) * 128].bitcast(f32r),
                        xh[bn][k4][hh][:].bitcast(f32r),
                        start=(k4 == 0), stop=(k4 == DC - 1),
                    )
                nc.scalar.copy(
                    gt_sb[bn][
                        :, m * N + hh * 512:m * N + (hh + 1) * 512
                    ].bitcast(f32r),
                    grp[:],
                )

            def v_group(bn, kc):
                grp = ps_ws.tile([128, 512], f32, tag="ws", name=f"pv{bn}{kc}")
                for k4 in range(DC):
                    nc.tensor.matmul(
                        grp[:],
                        xslice(bn, k4, kc).bitcast(f32r),
                        wvt_sb[k4][:].bitcast(f32r),
                        start=(k4 == 0), stop=(k4 == DC - 1),
                    )
                nc.scalar.copy(v_sb[bn][:, kc * D:(kc + 1) * D].bitcast(f32r), grp[:])

            def s_group(s, kc):
                b, h = s // 2, s % 2
                p_st = ps_ws.tile([128, 512], f32, tag="ws", name=f"st{s}_{kc}")
                for k4 in range(DC):
                    nc.tensor.matmul(
                        p_st[:],
                        xslice(b, k4, kc).bitcast(f32r),
                        gt_sb[b][
                            :, k4 * N + h * 512:k4 * N + (h + 1) * 512
                        ].bitcast(f32r),
                        start=(k4 == 0), stop=(k4 == DC - 1),
                    )
                e = epool.tile([128, 512], f32, tag=f"e{kc}", name=f"e{s}_{kc}")
                nc.scalar.activation(e[:].bitcast(f32r), p_st[:], Exp)
                e_sb[s][kc] = e
                ea = eapool.tile(
                    [128, 512], f32, tag=f"ea{kc % 2}", name=f"ea{s}_{kc}"
                )
                if kc == 0:
                    nc.vector.tensor_copy(ea[:].bitcast(f32r), e[:])
                else:
                    nc.vector.tensor_add(ea[:].bitcast(f32r), ea_fin[s][:], e[:])
                ea_fin[s] = ea

            def o_group(t, kc, lo=0, hi=512, tiles=None):
                """4 O^T matmuls (m inner) accumulating tile t, step kc,
                e-columns lo:hi."""
                bt = t // 2
                tiles = tiles if tiles is not None else p_ot[t]
                for m in range(DC):
                    nc.tensor.matmul(
                        tiles[m][:, 0:hi - lo],
                        v_sb[bt][
                            :, kc * D + m * 128:kc * D + (m + 1) * 128
                        ].bitcast(f32r),
                        e_sb[t][kc][:, lo:hi].bitcast(f32r),
                        start=(kc == 0), stop=(kc == KC - 1),
                    )

            def norm_chain(t, lo=0, hi=512, part=""):
                """denominator broadcast [i,q] = sum_k ea[k,q] via an
                all-ones stationary (fuses the row-sum and the rank-1
                broadcast into one matmul), then 1/x = exp(-ln(x))."""
                w = hi - lo
                p = ps_db.tile([128, 512], f32, tag="db", name=f"pbc{t}{part}")
                nc.tensor.matmul(
                    p[:, 0:w],
                    ones_mat[:].bitcast(f32r),
                    ea_fin[t][:, lo:hi].bitcast(f32r),
                    start=True, stop=True,
                )
                ln = rpool.tile([128, 512], f32, tag="ln", name=f"ln{t}{part}")
                nc.scalar.activation(ln[:, 0:w], p[:, 0:w], Ln)
                if t not in rc_sb:
                    rc_sb[t] = rpool.tile([128, 512], f32, tag="rc", name=f"rc{t}")
                nc.scalar.activation(rc_sb[t][:, lo:hi], ln[:, 0:w], Exp, scale=-1.0)

            def evict_ot(t, m, eng, lo=0, hi=512, tiles=None):
                w = hi - lo
                tiles = tiles if tiles is not None else p_ot[t]
                dst = otraw[t][:, m * 512 + lo:m * 512 + hi]
                if eng == "s":
                    nc.scalar.copy(dst, tiles[m][:, 0:w])
                else:
                    nc.vector.tensor_copy(dst, tiles[m][:, 0:w])

            def mul_ot(t, m, lo=0, hi=512):
                sl = otraw[t][:, m * 512 + lo:m * 512 + hi]
                nc.vector.tensor_mul(sl, sl, rc_sb[t][:, lo:hi])

            def dma_out(t, g, eng, lo=0, hi=512):
                """store one 256-row g-half of tile t (q-cols lo:hi)."""
                b, h = t // 2, t % 2
                dst = out_t[
                    b, g * 256:(g + 1) * 256, h * 512 + lo:h * 512 + hi
                ].rearrange("(m p) q -> p m q", p=128)
                q = nc.scalar if eng == "s" else nc.sync
                if lo == 0 and hi == 512:
                    q.dma_start(
                        dst,
                        otraw[t][:, 2 * g * 512:(2 * g + 2) * 512].rearrange(
                            "p (m q) -> p m q", m=2
                        ),
                    )
                    return
                for j in range(2):
                    m = 2 * g + j
                    q.dma_start(
                        dst[:, j:j + 1, :],
                        otraw[t][:, m * 512 + lo:m * 512 + hi].rearrange(
                            "p (m q) -> p m q", m=1
                        ),
                    )

            def alloc_slot(s):
                e_sb[s] = {}
                otraw[s] = opool.tile(
                    [128, DC * 512], f32, tag="otraw", name=f"orw{s}"
                )
                p_ot[s] = [
                    ps_ot.tile([128, 512], f32, tag=f"ot{m}", name=f"pot{s}{m}")
                    for m in range(DC)
                ]

            # ================= emission ================================
            load_xt(0)
            load_xt(1)

            for b in range(BPC):
                gt_sb[b] = gpool.tile([128, DC * N], f32, tag="gt", name=f"gt{b}")
                v_sb[b] = vpool.tile([128, KC * D], f32, tag="v", name=f"v{b}")

            # ---- prologue: gt(0) k4-outer across 8 psum groups
            # (3x ws + 1x db for hh=0; 4x ot for hh=1), consuming xt
            # chunks in DMA-arrival order.  S(0,h0) and v(0) interleave
            # into the DMA-chase gaps so the PE never idles long enough
            # to drop its p-state.
            grp_order = [(m, 0) for m in range(DC)] + [(m, 1) for m in range(DC)]
            grp = {}
            for i, (m, hh) in enumerate(grp_order):
                if i < 3:
                    grp[(m, hh)] = ps_ws.tile(
                        [128, 512], f32, tag="ws", name=f"pg0_{m}{hh}"
                    )
                elif i == 3:
                    grp[(m, hh)] = ps_db.tile(
                        [128, 512], f32, tag="db", name=f"pg0_{m}{hh}"
                    )
                else:
                    grp[(m, hh)] = ps_ot.tile(
                        [128, 512], f32, tag=f"ot{i - 4}", name=f"pg0_{m}{hh}"
                    )

            def gt0_round(k4, hh):
                for m in range(DC):
                    nc.tensor.matmul(
                        grp[(m, hh)][:],
                        a_sb[k4][:, m * 128:(m + 1) * 128].bitcast(f32r),
                        xh[0][k4][hh][:].bitcast(f32r),
                        start=(k4 == 0), stop=(k4 == DC - 1),
                    )

            def gt0_evict(hh):
                for m in range(DC):
                    dst = gt_sb[0][
                        :, m * N + hh * 512:m * N + (hh + 1) * 512
                    ].bitcast(f32r)
                    if m % 2 == 0:
                        nc.scalar.copy(dst, grp[(m, hh)][:])
                    else:
                        nc.vector.tensor_copy(dst, grp[(m, hh)][:])

            for k4 in range(DC):
                gt0_round(k4, 0)
            gt0_evict(0)
            alloc_slot(0)
            # hh=1 rounds chase the xt0-hh1 chunk arrivals; S(0,h0,kc<4)
            # (which needs only the hh=0 gt chunks) fills the gaps
            for kind, i in (("r", 0), ("r", 1), ("s", 0), ("r", 2),
                            ("s", 1), ("r", 3), ("s", 2), ("s", 3)):
                if kind == "r":
                    gt0_round(i, 1)
                else:
                    s_group(0, i)
            gt0_evict(1)
            # v(0) groups + the rest of S(0,h0)
            for kind, i in (("v", 0), ("v", 1), ("s", 4), ("v", 2),
                            ("s", 5), ("v", 3), ("s", 6), ("v", 4),
                            ("s", 7), ("v", 5), ("v", 6), ("v", 7)):
                if kind == "v":
                    v_group(0, i)
                else:
                    s_group(0, i)

            # ---- steady slots -----------------------------------------
            for s in range(1, NSLOT):
                b, h = s // 2, s % 2
                alloc_slot(s)
                if h == 1 and b + 2 < BPC:
                    load_xt(b + 2)

                for kc in range(KC):
                    if kc == 0 and s >= 2:
                        # free the ot psum banks first thing on Scalar/
                        # Vector so this slot's O groups are not blocked
                        evict_ot(s - 2, 0, "s")
                        evict_ot(s - 2, 1, "v")
                        evict_ot(s - 2, 2, "s")
                        evict_ot(s - 2, 3, "v")

                    # O leads at the last two steps so its psum stop (and
                    # the next slot's evictions) land earlier
                    if kc >= 6:
                        o_group(s - 1, kc)

                    s_group(s, kc)

                    if kc == 1:
                        # kc==1 keeps the norm matmul's counter-based waits
                        # clear of the slot-start Scalar/Vector backlog
                        norm_chain(s - 1)
                    if kc == 3 and s >= 2:
                        for m in range(DC):
                            mul_ot(s - 2, m)
                    if kc == 4 and s >= 2:
                        dma_out(s - 2, 0, "s")
                    if kc == 5 and s >= 2:
                        dma_out(s - 2, 1, "y")

                    if kc < 6:
                        o_group(s - 1, kc)

                    # gt(1) runs in slot 1 (xt(1) has fully landed by then)
                    if s == 1:
                        gt_group(1, kc % 4, kc // 4)
                        v_group(1, kc)
                    elif h == 0 and b >= 1 and b + 1 < BPC:
                        gt_group(b + 1, kc % 4, kc // 4)
                    elif h == 1 and s > 1 and b + 1 < BPC:
                        v_group(b + 1, kc)

            # ---- epilogue: tile L's O phase in two 256-col halves -----
            L = NSLOT - 1
            # free ot banks (tile L-2 was evicted in slot L; L-1 now)
            evict_ot(L - 1, 0, "s")
            evict_ot(L - 1, 1, "v")
            evict_ot(L - 1, 2, "s")
            evict_ot(L - 1, 3, "v")
            for kc in range(KC):
                o_group(L, kc, 0, 256)
                if kc == 1:
                    norm_chain(L, 0, 256, part="a")
                if kc == 2:
                    for m in range(DC):
                        mul_ot(L - 1, m)
                if kc == 3:
                    dma_out(L - 1, 0, "y")
                if kc == 4:
                    dma_out(L - 1, 1, "y")
            # half-a evictions free banks chunk by chunk for half-b
            for m in range(DC):
                evict_ot(L, m, "s" if m % 2 == 0 else "v", 0, 256)
            ot_b = [
                ps_ot.tile([128, 512], f32, tag=f"ot{m}", name=f"potb{m}")
                for m in range(DC)
            ]
            for kc in range(KC):
                o_group(L, kc, 256, 512, tiles=ot_b)
                if kc == 0:
                    norm_chain(L, 256, 512, part="b")
                if kc == 1:
                    for m in range(DC):
                        mul_ot(L, m, 0, 256)
                if kc == 3:
                    dma_out(L, 0, "s", 0, 256)
                if kc == 4:
                    dma_out(L, 1, "y", 0, 256)
            for m in range(DC):
                evict_ot(L, m, "s" if m % 2 == 0 else "v", 256, 512, tiles=ot_b)
            mul_ot(L, 0, 256, 512)
            mul_ot(L, 1, 256, 512)
            dma_out(L, 0, "s", 256, 512)
            mul_ot(L, 2, 256, 512)
            mul_ot(L, 3, 256, 512)
            dma_out(L, 1, "y", 256, 512)
    return nc


def _prepare_inputs(embeddings, Wq, Wk, Wv):
    xt_all = np.ascontiguousarray(embeddings.transpose(0, 2, 1)).astype(
        np.float32, copy=False
    )
    a_mat = (
        Wq.astype(np.float64).T @ Wk.astype(np.float64) / np.sqrt(float(D))
    ).astype(np.float32)
    wvt = np.ascontiguousarray(Wv.T).astype(np.float32, copy=False)
    ones_mat = np.ones((128, 128), np.float32)
    in_maps = []
    for i in range(NCORES):
        in_maps.append(
            {
                "xt": np.ascontiguousarray(xt_all[i * BPC:(i + 1) * BPC]),
                "a_mat": a_mat,
                "wvt": wvt,
                "ones_mat": ones_mat,
            }
        )
    return in_maps


def _get_nc():
    if "nc" not in _cache:
        nc = _build()
        _split_sync_waits(nc)
        _cache["nc"] = nc
    return _cache["nc"]


def _assemble(results):
    out = np.empty((B, N, D), np.float32)
    for i in range(NCORES):
        ot = results[i]["out_t"]  # [BPC, D, N]
        out[i * BPC:(i + 1) * BPC] = ot.transpose(0, 2, 1)
    return out


def kernel(embeddings, Wq, Wk, Wv):
    from concourse.bass_utils import run_bass_kernel_spmd

    embeddings = np.asarray(embeddings, dtype=np.float32)
    in_maps = _prepare_inputs(
        embeddings, np.asarray(Wq), np.asarray(Wk), np.asarray(Wv)
    )
    res = run_bass_kernel_spmd(_get_nc(), in_maps, list(range(NCORES)))
    return _assemble(res.results)


# revision 10
# speedup vs baseline: 1.0781x; 1.0059x over previous
"""Trainium2 Bass kernel for nn_Attention: single-head attention,
B=32, N=1024, DIM=512, fp32.

    q = X @ Wq.T ; k = X @ Wk.T ; v = X @ Wv.T
    out = softmax((q k^T)/sqrt(D)) @ v

Strategy (8 NeuronCores, data-parallel over batch, 4 batches/core):
  - Host folds A = (Wq.T @ Wk)/sqrt(D)  so scores = X A X.T  — saves one
    projection-sized matmul per batch and needs only X (transposed) on
    device.
  - All tensors live transposed on device: XT [d, n], GT = (X A).T,
    V [n, e], scores ST [k, q] (k on partitions).  Softmax runs along
    the partition axis: exp on ScalarE, partition sums via a ones-vector
    matmul, broadcast of 1/denom via a rank-1 ones matmul, normalization
    on VectorE.  Attention output is produced transposed (OT [e, q]) and
    the host transposes it back.
  - Matmuls use the float32r PE mode (full-rate fp32 streaming).
  - Flat software pipeline over h-tiles s = 2b+h (512 q-cols each):
    slot s interleaves S(s) matmuls with O(s-1) matmuls and the gt/v
    projection matmuls of batch b+1; evict/normalize/store of tile s-2
    ride along on Scalar/Vector/DMA.  PE bubbles also reset the PE
    p-state (~3us of half-clock each), so the stream must stay dense.
    The last tile's O phase runs as two 256-wide halves so its norm
    chain overlaps the second half.
"""
import numpy as np

B, N, D = 32, 1024, 512
NCORES = 8
BPC = B // NCORES          # batches per core
DC = D // 128              # 4 chunks of 128 along d / e
KC = N // 128              # 8 chunks of 128 along k
NSLOT = 2 * BPC            # h-tiles per core (512 q-cols each)

_cache = {}


def _split_sync_waits(nc):
    """walrus on this image accepts at most ONE semaphore wait per
    instruction; hoist extras onto InstNoOp carriers on the same engine
    (same-engine program order preserves the gating)."""
    import concourse.mybir as mybir

    ctr = 0
    for f in nc.m.functions:
        for bb in f.blocks:
            out = []
            changed = False
            for ins in bb.instructions:
                si = getattr(ins, "sync_info", None)
                waits = list(si.on_wait) if si and si.on_wait else []
                if len(waits) > 1:
                    for w in waits[:-1]:
                        ctr += 1
                        out.append(
                            mybir.InstNoOp(
                                name=f"wsplit-{ctr}",
                                engine=ins.engine,
                                bass_nofuse=True,
                                sync_info=mybir.SyncInfo(on_wait=[w], on_update=[]),
                            )
                        )
                    ins.sync_info = mybir.SyncInfo(
                        on_wait=waits[-1:], on_update=list(si.on_update or [])
                    )
                    changed = True
                out.append(ins)
            if changed:
                bb.instructions[:] = out


def _build():
    import concourse.bass as bass
    import concourse.mybir as mybir
    import concourse.tile as tile

    f32 = mybir.dt.float32
    f32r = mybir.dt.float32r
    Exp = mybir.ActivationFunctionType.Exp
    Ln = mybir.ActivationFunctionType.Ln

    nc = bass.Bass(target_bir_lowering=False)

    xt = nc.dram_tensor("xt", [BPC, D, N], f32, kind="ExternalInput")
    a_mat = nc.dram_tensor("a_mat", [D, D], f32, kind="ExternalInput")
    wvt = nc.dram_tensor("wvt", [D, D], f32, kind="ExternalInput")
    ones_d = nc.dram_tensor("ones_mat", [128, 128], f32, kind="ExternalInput")
    out_t = nc.dram_tensor("out_t", [BPC, D, N], f32, kind="ExternalOutput")

    with tile.TileContext(nc) as tc:
        with (
            tc.tile_pool(name="wpool", bufs=1) as wpool,
            tc.tile_pool(name="xpool", bufs=3) as xpool,
            tc.tile_pool(name="gpool", bufs=2) as gpool,
            tc.tile_pool(name="vpool", bufs=2) as vpool,
            tc.tile_pool(name="epool", bufs=2) as epool,
            tc.tile_pool(name="eapool", bufs=2) as eapool,
            tc.tile_pool(name="opool", bufs=2) as opool,
            tc.tile_pool(name="rpool", bufs=2) as rpool,
            tc.tile_pool(name="ps_ws", bufs=3, space="PSUM") as ps_ws,
            tc.tile_pool(name="ps_ot", bufs=1, space="PSUM") as ps_ot,
            tc.tile_pool(name="ps_db", bufs=1, space="PSUM") as ps_db,
        ):
            # ---------------- weights / constants (sync queue) ----------
            a_sb = []
            for c in range(DC):
                t = wpool.tile([128, D], f32, tag=f"a{c}", name=f"a_sb{c}")
                nc.sync.dma_start(t[:].bitcast(f32r), a_mat[c * 128:(c + 1) * 128, :].bitcast(f32r))
                a_sb.append(t)
            ones_mat = wpool.tile([128, 128], f32, tag="ones")
            nc.sync.dma_start(ones_mat[:].bitcast(f32r), ones_d[:].bitcast(f32r))
            wvt_sb = [
                wpool.tile([128, D], f32, tag=f"wvt{c}", name=f"wvt_sb{c}")
                for c in range(DC)
            ]

            def load_wvt():
                # rides the scalar queue behind xt0-hh0 so it lands by
                # ~20us for the prologue v(0) groups
                for c in range(DC):
                    nc.scalar.dma_start(
                        wvt_sb[c][:].bitcast(f32r),
                        wvt[c * 128:(c + 1) * 128, :].bitcast(f32r),
                    )

            # ---------------- xt half-tiles (scalar queue) --------------
            # xh[b][k4][hh] = X^T[d-chunk k4, n-half hh]  [128, 512]
            xh = {}

            def load_xt(b):
                tiles = []
                for k4 in range(DC):
                    tiles.append(
                        [
                            xpool.tile(
                                [128, 512], f32, tag=f"x{k4}{hh}",
                                name=f"x_b{b}_{k4}{hh}",
                            )
                            for hh in range(2)
                        ]
                    )
                # hh-outer issue order so the hh=0 set lands first;
                # the two halves ride different queues so they stream in
                # parallel (b=0: hh0 scalar / hh1 sync; b>=1 reversed)
                for hh in range(2):
                    for k4 in range(DC):
                        q = nc.scalar if (b == 0) == (hh == 0) else nc.sync
                        q.dma_start(
                            tiles[k4][hh][:].bitcast(f32r),
                            xt[
                                b, k4 * 128:(k4 + 1) * 128,
                                hh * 512:(hh + 1) * 512,
                            ].bitcast(f32r),
                        )
                xh[b] = tiles

            # xt stationary slice (column chunk kc of N)
            def xslice(b, k4, kc):
                return xh[b][k4][kc // 4][:, (kc % 4) * 128:((kc % 4) + 1) * 128]

            # ---------------- per-batch / per-slot state ----------------
            gt_sb = {}   # [128, DC*N]  G^T chunks at cols m*N + hh*512
            v_sb = {}    # [128, KC*D]  V chunks at cols kc*D
            e_sb = {}    # e_sb[s][kc]  exp tiles [128, 512]
            ea_fin = {}  # final running-sum tile per slot
            rc_sb = {}   # [128, 512] 1/denom broadcast
            otraw = {}   # [128, DC*512] raw O^T per slot
            p_ot = {}    # p_ot[s] = 4 psum accumulators

            # ---------------- emitters ----------------------------------
            def gt_group(bn, m, hh):
                grp = ps_ws.tile([128, 512], f32, tag="ws", name=f"pg{bn}{m}{hh}")
                for k4 in range(DC):
                    nc.tensor.matmul(
                        grp[:],
                        a_sb[k4][:, m * 128:(m + 1) * 128].bitcast(f32r),
                        xh[bn][k4][hh][:].bitcast(f32r),
                        start=(k4 == 0), stop=(k4 == DC - 1),
                    )
                nc.scalar.copy(
                    gt_sb[bn][
                        :, m * N + hh * 512:m * N + (hh + 1) * 512
                    ].bitcast(f32r),
                    grp[:],
                )

            def v_group(bn, kc):
                grp = ps_ws.tile([128, 512], f32, tag="ws", name=f"pv{bn}{kc}")
                for k4 in range(DC):
                    nc.tensor.matmul(
                        grp[:],
                        xslice(bn, k4, kc).bitcast(f32r),
                        wvt_sb[k4][:].bitcast(f32r),
                        start=(k4 == 0), stop=(k4 == DC - 1),
                    )
                nc.scalar.copy(v_sb[bn][:, kc * D:(kc + 1) * D].bitcast(f32r), grp[:])

            def s_group(s, kc):
                b, h = s // 2, s % 2
                p_st = ps_ws.tile([128, 512], f32, tag="ws", name=f"st{s}_{kc}")
                for k4 in range(DC):
                    nc.tensor.matmul(
                        p_st[:],
                        xslice(b, k4, kc).bitcast(f32r),
                        gt_sb[b][
                            :, k4 * N + h * 512:k4 * N + (h + 1) * 512
                        ].bitcast(f32r),
                        start=(k4 == 0), stop=(k4 == DC - 1),
                    )
                e = epool.tile([128, 512], f32, tag=f"e{kc}", name=f"e{s}_{kc}")
                nc.scalar.activation(e[:].bitcast(f32r), p_st[:], Exp)
                e_sb[s][kc] = e
                ea = eapool.tile(
                    [128, 512], f32, tag=f"ea{kc % 2}", name=f"ea{s}_{kc}"
                )
                if kc == 0:
                    nc.vector.tensor_copy(ea[:].bitcast(f32r), e[:])
                else:
                    nc.vector.tensor_add(ea[:].bitcast(f32r), ea_fin[s][:], e[:])
                ea_fin[s] = ea

            def o_group(t, kc, lo=0, hi=512, tiles=None):
                """4 O^T matmuls (m inner) accumulating tile t, step kc,
                e-columns lo:hi."""
                bt = t // 2
                tiles = tiles if tiles is not None else p_ot[t]
                for m in range(DC):
                    nc.tensor.matmul(
                        tiles[m][:, 0:hi - lo],
                        v_sb[bt][
                            :, kc * D + m * 128:kc * D + (m + 1) * 128
                        ].bitcast(f32r),
                        e_sb[t][kc][:, lo:hi].bitcast(f32r),
                        start=(kc == 0), stop=(kc == KC - 1),
                    )

            def norm_chain(t, lo=0, hi=512, part=""):
                """denominator broadcast [i,q] = sum_k ea[k,q] via an
                all-ones stationary (fuses the row-sum and the rank-1
                broadcast into one matmul), then 1/x = exp(-ln(x))."""
                w = hi - lo
                p = ps_db.tile([128, 512], f32, tag="db", name=f"pbc{t}{part}")
                nc.tensor.matmul(
                    p[:, 0:w],
                    ones_mat[:].bitcast(f32r),
                    ea_fin[t][:, lo:hi].bitcast(f32r),
                    start=True, stop=True,
                )
                ln = rpool.tile([128, 512], f32, tag="ln", name=f"ln{t}{part}")
                nc.scalar.activation(ln[:, 0:w], p[:, 0:w], Ln)
                if t not in rc_sb:
                    rc_sb[t] = rpool.tile([128, 512], f32, tag="rc", name=f"rc{t}")
                nc.scalar.activation(rc_sb[t][:, lo:hi], ln[:, 0:w], Exp, scale=-1.0)

            def evict_ot(t, m, eng, lo=0, hi=512, tiles=None):
                w = hi - lo
                tiles = tiles if tiles is not None else p_ot[t]
                dst = otraw[t][:, m * 512 + lo:m * 512 + hi]
                if eng == "s":
                    nc.scalar.copy(dst, tiles[m][:, 0:w])
                else:
                    nc.vector.tensor_copy(dst, tiles[m][:, 0:w])

            def mul_ot(t, m, lo=0, hi=512):
                sl = otraw[t][:, m * 512 + lo:m * 512 + hi]
                nc.vector.tensor_mul(sl, sl, rc_sb[t][:, lo:hi])

            def dma_out(t, g, eng, lo=0, hi=512):
                """store one 256-row g-half of tile t (q-cols lo:hi)."""
                b, h = t // 2, t % 2
                dst = out_t[
                    b, g * 256:(g + 1) * 256, h * 512 + lo:h * 512 + hi
                ].rearrange("(m p) q -> p m q", p=128)
                q = nc.scalar if eng == "s" else nc.sync
                if lo == 0 and hi == 512:
                    q.dma_start(
                        dst,
                        otraw[t][:, 2 * g * 512:(2 * g + 2) * 512].rearrange(
                            "p (m q) -> p m q", m=2
                        ),
                    )
                    return
                for j in range(2):
                    m = 2 * g + j
                    q.dma_start(
                        dst[:, j:j + 1, :],
                        otraw[t][:, m * 512 + lo:m * 512 + hi].rearrange(
                            "p (m q) -> p m q", m=1
                        ),
                    )

            def alloc_slot(s):
                e_sb[s] = {}
                otraw[s] = opool.tile(
                    [128, DC * 512], f32, tag="otraw", name=f"orw{s}"
                )
                p_ot[s] = [
                    ps_ot.tile([128, 512], f32, tag=f"ot{m}", name=f"pot{s}{m}")
                    for m in range(DC)
                ]

            # ================= emission ================================
            load_xt(0)
            load_wvt()
            load_xt(1)

            for b in range(BPC):
                gt_sb[b] = gpool.tile([128, DC * N], f32, tag="gt", name=f"gt{b}")
                v_sb[b] = vpool.tile([128, KC * D], f32, tag="v", name=f"v{b}")

            # ---- prologue: gt(0) k4-outer across 8 psum groups
            # (3x ws + 1x db for hh=0; 4x ot for hh=1), consuming xt
            # chunks in DMA-arrival order.  S(0,h0) and v(0) interleave
            # into the DMA-chase gaps so the PE never idles long enough
            # to drop its p-state.
            grp_order = [(m, 0) for m in range(DC)] + [(m, 1) for m in range(DC)]
            grp = {}
            for i, (m, hh) in enumerate(grp_order):
                if i < 3:
                    grp[(m, hh)] = ps_ws.tile(
                        [128, 512], f32, tag="ws", name=f"pg0_{m}{hh}"
                    )
                elif i == 3:
                    grp[(m, hh)] = ps_db.tile(
                        [128, 512], f32, tag="db", name=f"pg0_{m}{hh}"
                    )
                else:
                    grp[(m, hh)] = ps_ot.tile(
                        [128, 512], f32, tag=f"ot{i - 4}", name=f"pg0_{m}{hh}"
                    )

            def gt0_round(k4, hh):
                for m in range(DC):
                    nc.tensor.matmul(
                        grp[(m, hh)][:],
                        a_sb[k4][:, m * 128:(m + 1) * 128].bitcast(f32r),
                        xh[0][k4][hh][:].bitcast(f32r),
                        start=(k4 == 0), stop=(k4 == DC - 1),
                    )

            def gt0_evict(hh):
                for m in range(DC):
                    dst = gt_sb[0][
                        :, m * N + hh * 512:m * N + (hh + 1) * 512
                    ].bitcast(f32r)
                    if m % 2 == 0:
                        nc.scalar.copy(dst, grp[(m, hh)][:])
                    else:
                        nc.vector.tensor_copy(dst, grp[(m, hh)][:])

            for k4 in range(DC):
                gt0_round(k4, 0)
            gt0_evict(0)
            alloc_slot(0)
            # hh=1 rounds chase the xt0-hh1 chunk arrivals; S(0,h0,kc<4)
            # (which needs only the hh=0 gt chunks) fills the gaps
            for kind, i in (("r", 0), ("r", 1), ("s", 0), ("r", 2),
                            ("s", 1), ("r", 3), ("s", 2), ("s", 3)):
                if kind == "r":
                    gt0_round(i, 1)
                else:
                    s_group(0, i)
            gt0_evict(1)
            # v(0) groups + the rest of S(0,h0)
            for kind, i in (("v", 0), ("v", 1), ("v", 2), ("s", 4),
                            ("v", 3), ("s", 5), ("v", 4), ("s", 6),
                            ("v", 5), ("s", 7), ("v", 6), ("v", 7)):
                if kind == "v":
                    v_group(0, i)
                else:
                    s_group(0, i)

            # ---- steady slots -----------------------------------------
            for s in range(1, NSLOT):
                b, h = s // 2, s % 2
                alloc_slot(s)
                if h == 1 and b + 2 < BPC:
                    load_xt(b + 2)

                for kc in range(KC):
                    if kc == 0 and s >= 2:
                        # free the ot psum banks first thing on Scalar/
                        # Vector so this slot's O groups are not blocked
                        evict_ot(s - 2, 0, "s")
                        evict_ot(s - 2, 1, "v")
                        evict_ot(s - 2, 2, "s")
                        evict_ot(s - 2, 3, "v")

                    # O leads at the last two steps so its psum stop (and
                    # the next slot's evictions) land earlier
                    if kc >= 6:
                        o_group(s - 1, kc)

                    s_group(s, kc)

                    if kc == 1:
                        # kc==1 keeps the norm matmul's counter-based waits
                        # clear of the slot-start Scalar/Vector backlog
                        norm_chain(s - 1)
                    if kc == 3 and s >= 2:
                        for m in range(DC):
                            mul_ot(s - 2, m)
                    if kc == 4 and s >= 2:
                        dma_out(s - 2, 0, "s")
                    if kc == 5 and s >= 2:
                        dma_out(s - 2, 1, "y")

                    if kc < 6:
                        o_group(s - 1, kc)

                    # gt(1) runs in slot 1 (xt(1) has fully landed by then)
                    if s == 1:
                        gt_group(1, kc % 4, kc // 4)
                        v_group(1, kc)
                    elif h == 0 and b >= 1 and b + 1 < BPC:
                        gt_group(b + 1, kc % 4, kc // 4)
                    elif h == 1 and s > 1 and b + 1 < BPC:
                        v_group(b + 1, kc)

            # ---- epilogue: tile L's O phase in two 256-col halves -----
            L = NSLOT - 1
            # free ot banks (tile L-2 was evicted in slot L; L-1 now)
            evict_ot(L - 1, 0, "s")
            evict_ot(L - 1, 1, "v")
            evict_ot(L - 1, 2, "s")
            evict_ot(L - 1, 3, "v")
            for kc in range(KC):
                o_group(L, kc, 0, 256)
                if kc == 1:
                    norm_chain(L, 0, 256, part="a")
                if kc == 2:
                    for m in range(DC):
                        mul_ot(L - 1, m)
                if kc == 3:
                    dma_out(L - 1, 0, "y")
                if kc == 4:
                    dma_out(L - 1, 1, "y")
            # half-a evictions free banks chunk by chunk for half-b
            for m in range(DC):
                evict_ot(L, m, "s" if m % 2 == 0 else "v", 0, 256)
            ot_b = [
                ps_ot.tile([128, 512], f32, tag=f"ot{m}", name=f"potb{m}")
                for m in range(DC)
            ]
            for kc in range(KC):
                o_group(L, kc, 256, 512, tiles=ot_b)
                if kc == 0:
                    norm_chain(L, 256, 512, part="b")
                if kc == 1:
                    for m in range(DC):
                        mul_ot(L, m, 0, 256)
                if kc == 3:
                    dma_out(L, 0, "s", 0, 256)
                if kc == 4:
                    dma_out(L, 1, "y", 0, 256)
            for m in range(DC):
                evict_ot(L, m, "s" if m % 2 == 0 else "v", 256, 512, tiles=ot_b)
            for m in range(DC):
                mul_ot(L, m, 256, 512)
                g, j = m // 2, m % 2
                q = nc.scalar if m % 2 == 0 else nc.sync
                dst = out_t[
                    L // 2, g * 256:(g + 1) * 256, (L % 2) * 512 + 256:(L % 2) * 512 + 512
                ].rearrange("(m p) q -> p m q", p=128)
                q.dma_start(
                    dst[:, j:j + 1, :],
                    otraw[L][:, m * 512 + 256:m * 512 + 512].rearrange(
                        "p (m q) -> p m q", m=1
                    ),
                )
    return nc


def _prepare_inputs(embeddings, Wq, Wk, Wv):
    xt_all = np.ascontiguousarray(embeddings.transpose(0, 2, 1)).astype(
        np.float32, copy=False
    )
    a_mat = (
        Wq.astype(np.float64).T @ Wk.astype(np.float64) / np.sqrt(float(D))
    ).astype(np.float32)
    wvt = np.ascontiguousarray(Wv.T).astype(np.float32, copy=False)
    ones_mat = np.ones((128, 128), np.float32)
    in_maps = []
    for i in range(NCORES):
        in_maps.append(
            {
                "xt": np.ascontiguousarray(xt_all[i * BPC:(i + 1) * BPC]),
                "a_mat": a_mat,
                "wvt": wvt,
                "ones_mat": ones_mat,
            }
        )
    return in_maps


def _get_nc():
    if "nc" not in _cache:
        nc = _build()
        _split_sync_waits(nc)
        _cache["nc"] = nc
    return _cache["nc"]


def _assemble(results):
    out = np.empty((B, N, D), np.float32)
    for i in range(NCORES):
        ot = results[i]["out_t"]  # [BPC, D, N]
        out[i * BPC:(i + 1) * BPC] = ot.transpose(0, 2, 1)
    return out


def kernel(embeddings, Wq, Wk, Wv):
    from concourse.bass_utils import run_bass_kernel_spmd

    embeddings = np.asarray(embeddings, dtype=np.float32)
    in_maps = _prepare_inputs(
        embeddings, np.asarray(Wq), np.asarray(Wk), np.asarray(Wv)
    )
    res = run_bass_kernel_spmd(_get_nc(), in_maps, list(range(NCORES)))
    return _assemble(res.results)


# revision 11
# speedup vs baseline: 1.0883x; 1.0095x over previous
"""Trainium2 Bass kernel for nn_Attention: single-head attention,
B=32, N=1024, DIM=512, fp32.

    q = X @ Wq.T ; k = X @ Wk.T ; v = X @ Wv.T
    out = softmax((q k^T)/sqrt(D)) @ v

Strategy (8 NeuronCores, data-parallel over batch, 4 batches/core):
  - Host folds A = (Wq.T @ Wk)/sqrt(D)  so scores = X A X.T  — saves one
    projection-sized matmul per batch and needs only X (transposed) on
    device.
  - All tensors live transposed on device: XT [d, n], GT = (X A).T,
    V [n, e], scores ST [k, q] (k on partitions).  Softmax runs along
    the partition axis: exp on ScalarE, partition sums via a ones-vector
    matmul, broadcast of 1/denom via a rank-1 ones matmul, normalization
    on VectorE.  Attention output is produced transposed (OT [e, q]) and
    the host transposes it back.
  - Matmuls use the float32r PE mode (full-rate fp32 streaming).
  - Flat software pipeline over h-tiles s = 2b+h (512 q-cols each):
    slot s interleaves S(s) matmuls with O(s-1) matmuls and the gt/v
    projection matmuls of batch b+1; evict/normalize/store of tile s-2
    ride along on Scalar/Vector/DMA.  PE bubbles also reset the PE
    p-state (~3us of half-clock each), so the stream must stay dense.
    The last tile's O phase runs as two 256-wide halves so its norm
    chain overlaps the second half.
"""
import numpy as np

B, N, D = 32, 1024, 512
NCORES = 8
BPC = B // NCORES          # batches per core
DC = D // 128              # 4 chunks of 128 along d / e
KC = N // 128              # 8 chunks of 128 along k
NSLOT = 2 * BPC            # h-tiles per core (512 q-cols each)

_cache = {}


def _split_sync_waits(nc):
    """walrus on this image accepts at most ONE semaphore wait per
    instruction; hoist extras onto InstNoOp carriers on the same engine
    (same-engine program order preserves the gating)."""
    import concourse.mybir as mybir

    ctr = 0
    for f in nc.m.functions:
        for bb in f.blocks:
            out = []
            changed = False
            for ins in bb.instructions:
                si = getattr(ins, "sync_info", None)
                waits = list(si.on_wait) if si and si.on_wait else []
                if len(waits) > 1:
                    for w in waits[:-1]:
                        ctr += 1
                        out.append(
                            mybir.InstNoOp(
                                name=f"wsplit-{ctr}",
                                engine=ins.engine,
                                bass_nofuse=True,
                                sync_info=mybir.SyncInfo(on_wait=[w], on_update=[]),
                            )
                        )
                    ins.sync_info = mybir.SyncInfo(
                        on_wait=waits[-1:], on_update=list(si.on_update or [])
                    )
                    changed = True
                out.append(ins)
            if changed:
                bb.instructions[:] = out


def _build():
    import concourse.bass as bass
    import concourse.mybir as mybir
    import concourse.tile as tile

    f32 = mybir.dt.float32
    f32r = mybir.dt.float32r
    Exp = mybir.ActivationFunctionType.Exp
    Ln = mybir.ActivationFunctionType.Ln

    nc = bass.Bass(target_bir_lowering=False)

    xt = nc.dram_tensor("xt", [BPC, D, N], f32, kind="ExternalInput")
    a_mat = nc.dram_tensor("a_mat", [D, D], f32, kind="ExternalInput")
    wvt = nc.dram_tensor("wvt", [D, D], f32, kind="ExternalInput")
    ones_d = nc.dram_tensor("ones_mat", [128, 128], f32, kind="ExternalInput")
    out_t = nc.dram_tensor("out_t", [BPC, D, N], f32, kind="ExternalOutput")

    with tile.TileContext(nc) as tc:
        with (
            tc.tile_pool(name="wpool", bufs=1) as wpool,
            tc.tile_pool(name="xpool", bufs=3) as xpool,
            tc.tile_pool(name="gpool", bufs=2) as gpool,
            tc.tile_pool(name="vpool", bufs=2) as vpool,
            tc.tile_pool(name="epool", bufs=2) as epool,
            tc.tile_pool(name="eapool", bufs=2) as eapool,
            tc.tile_pool(name="opool", bufs=2) as opool,
            tc.tile_pool(name="rpool", bufs=2) as rpool,
            tc.tile_pool(name="ps_ws", bufs=3, space="PSUM") as ps_ws,
            tc.tile_pool(name="ps_ot", bufs=1, space="PSUM") as ps_ot,
            tc.tile_pool(name="ps_db", bufs=1, space="PSUM") as ps_db,
        ):
            # ---------------- weights / constants (sync queue) ----------
            a_sb = []
            for c in range(DC):
                t = wpool.tile([128, D], f32, tag=f"a{c}", name=f"a_sb{c}")
                nc.sync.dma_start(t[:].bitcast(f32r), a_mat[c * 128:(c + 1) * 128, :].bitcast(f32r))
                a_sb.append(t)
            ones_mat = wpool.tile([128, 128], f32, tag="ones")
            nc.sync.dma_start(ones_mat[:].bitcast(f32r), ones_d[:].bitcast(f32r))
            wvt_sb = [
                wpool.tile([128, D], f32, tag=f"wvt{c}", name=f"wvt_sb{c}")
                for c in range(DC)
            ]

            def load_wvt():
                # rides the scalar queue behind xt0-hh0 so it lands by
                # ~20us for the prologue v(0) groups
                for c in range(DC):
                    nc.scalar.dma_start(
                        wvt_sb[c][:].bitcast(f32r),
                        wvt[c * 128:(c + 1) * 128, :].bitcast(f32r),
                    )

            # ---------------- xt half-tiles (scalar queue) --------------
            # xh[b][k4][hh] = X^T[d-chunk k4, n-half hh]  [128, 512]
            xh = {}

            def load_xt(b):
                tiles = []
                for k4 in range(DC):
                    tiles.append(
                        [
                            xpool.tile(
                                [128, 512], f32, tag=f"x{k4}{hh}",
                                name=f"x_b{b}_{k4}{hh}",
                            )
                            for hh in range(2)
                        ]
                    )
                # hh-outer issue order so the hh=0 set lands first;
                # the two halves ride different queues so they stream in
                # parallel (b=0: hh0 scalar / hh1 sync; b>=1 reversed)
                for hh in range(2):
                    for k4 in range(DC):
                        q = nc.scalar if (b == 0) == (hh == 0) else nc.sync
                        q.dma_start(
                            tiles[k4][hh][:].bitcast(f32r),
                            xt[
                                b, k4 * 128:(k4 + 1) * 128,
                                hh * 512:(hh + 1) * 512,
                            ].bitcast(f32r),
                        )
                xh[b] = tiles

            # xt stationary slice (column chunk kc of N)
            def xslice(b, k4, kc):
                return xh[b][k4][kc // 4][:, (kc % 4) * 128:((kc % 4) + 1) * 128]

            # ---------------- per-batch / per-slot state ----------------
            gt_sb = {}   # [128, DC*N]  G^T chunks at cols m*N + hh*512
            v_sb = {}    # [128, KC*D]  V chunks at cols kc*D
            e_sb = {}    # e_sb[s][kc]  exp tiles [128, 512]
            ea_fin = {}  # final running-sum tile per slot
            rc_sb = {}   # [128, 512] 1/denom broadcast
            otraw = {}   # [128, DC*512] raw O^T per slot
            p_ot = {}    # p_ot[s] = 4 psum accumulators

            # ---------------- emitters ----------------------------------
            def gt_group(bn, m, hh):
                grp = ps_ws.tile([128, 512], f32, tag="ws", name=f"pg{bn}{m}{hh}")
                for k4 in range(DC):
                    nc.tensor.matmul(
                        grp[:],
                        a_sb[k4][:, m * 128:(m + 1) * 128].bitcast(f32r),
                        xh[bn][k4][hh][:].bitcast(f32r),
                        start=(k4 == 0), stop=(k4 == DC - 1),
                    )
                nc.scalar.copy(
                    gt_sb[bn][
                        :, m * N + hh * 512:m * N + (hh + 1) * 512
                    ].bitcast(f32r),
                    grp[:],
                )

            def v_group(bn, kc):
                grp = ps_ws.tile([128, 512], f32, tag="ws", name=f"pv{bn}{kc}")
                for k4 in range(DC):
                    nc.tensor.matmul(
                        grp[:],
                        xslice(bn, k4, kc).bitcast(f32r),
                        wvt_sb[k4][:].bitcast(f32r),
                        start=(k4 == 0), stop=(k4 == DC - 1),
                    )
                nc.scalar.copy(v_sb[bn][:, kc * D:(kc + 1) * D].bitcast(f32r), grp[:])

            def s_group(s, kc, pool=None, tag="ws"):
                b, h = s // 2, s % 2
                p_st = (pool or ps_ws).tile(
                    [128, 512], f32, tag=tag, name=f"st{s}_{kc}"
                )
                for k4 in range(DC):
                    nc.tensor.matmul(
                        p_st[:],
                        xslice(b, k4, kc).bitcast(f32r),
                        gt_sb[b][
                            :, k4 * N + h * 512:k4 * N + (h + 1) * 512
                        ].bitcast(f32r),
                        start=(k4 == 0), stop=(k4 == DC - 1),
                    )
                e = epool.tile([128, 512], f32, tag=f"e{kc}", name=f"e{s}_{kc}")
                nc.scalar.activation(e[:].bitcast(f32r), p_st[:], Exp)
                e_sb[s][kc] = e
                ea = eapool.tile(
                    [128, 512], f32, tag=f"ea{kc % 2}", name=f"ea{s}_{kc}"
                )
                if kc == 0:
                    nc.vector.tensor_copy(ea[:].bitcast(f32r), e[:])
                else:
                    nc.vector.tensor_add(ea[:].bitcast(f32r), ea_fin[s][:], e[:])
                ea_fin[s] = ea

            def o_group(t, kc, lo=0, hi=512, tiles=None):
                """4 O^T matmuls (m inner) accumulating tile t, step kc,
                e-columns lo:hi."""
                bt = t // 2
                tiles = tiles if tiles is not None else p_ot[t]
                for m in range(DC):
                    nc.tensor.matmul(
                        tiles[m][:, 0:hi - lo],
                        v_sb[bt][
                            :, kc * D + m * 128:kc * D + (m + 1) * 128
                        ].bitcast(f32r),
                        e_sb[t][kc][:, lo:hi].bitcast(f32r),
                        start=(kc == 0), stop=(kc == KC - 1),
                    )

            def norm_chain(t, lo=0, hi=512, part=""):
                """denominator broadcast [i,q] = sum_k ea[k,q] via an
                all-ones stationary (fuses the row-sum and the rank-1
                broadcast into one matmul), then 1/x = exp(-ln(x))."""
                w = hi - lo
                p = ps_db.tile([128, 512], f32, tag="db", name=f"pbc{t}{part}")
                nc.tensor.matmul(
                    p[:, 0:w],
                    ones_mat[:].bitcast(f32r),
                    ea_fin[t][:, lo:hi].bitcast(f32r),
                    start=True, stop=True,
                )
                ln = rpool.tile([128, 512], f32, tag="ln", name=f"ln{t}{part}")
                nc.scalar.activation(ln[:, 0:w], p[:, 0:w], Ln)
                if t not in rc_sb:
                    rc_sb[t] = rpool.tile([128, 512], f32, tag="rc", name=f"rc{t}")
                nc.scalar.activation(rc_sb[t][:, lo:hi], ln[:, 0:w], Exp, scale=-1.0)

            def evict_ot(t, m, eng, lo=0, hi=512, tiles=None):
                w = hi - lo
                tiles = tiles if tiles is not None else p_ot[t]
                dst = otraw[t][:, m * 512 + lo:m * 512 + hi]
                if eng == "s":
                    nc.scalar.copy(dst, tiles[m][:, 0:w])
                else:
                    nc.vector.tensor_copy(dst, tiles[m][:, 0:w])

            def mul_ot(t, m, lo=0, hi=512):
                sl = otraw[t][:, m * 512 + lo:m * 512 + hi]
                nc.vector.tensor_mul(sl, sl, rc_sb[t][:, lo:hi])

            def dma_out(t, g, eng, lo=0, hi=512):
                """store one 256-row g-half of tile t (q-cols lo:hi)."""
                b, h = t // 2, t % 2
                dst = out_t[
                    b, g * 256:(g + 1) * 256, h * 512 + lo:h * 512 + hi
                ].rearrange("(m p) q -> p m q", p=128)
                q = nc.scalar if eng == "s" else nc.sync
                if lo == 0 and hi == 512:
                    q.dma_start(
                        dst,
                        otraw[t][:, 2 * g * 512:(2 * g + 2) * 512].rearrange(
                            "p (m q) -> p m q", m=2
                        ),
                    )
                    return
                for j in range(2):
                    m = 2 * g + j
                    q.dma_start(
                        dst[:, j:j + 1, :],
                        otraw[t][:, m * 512 + lo:m * 512 + hi].rearrange(
                            "p (m q) -> p m q", m=1
                        ),
                    )

            def alloc_slot(s):
                e_sb[s] = {}
                otraw[s] = opool.tile(
                    [128, DC * 512], f32, tag="otraw", name=f"orw{s}"
                )
                p_ot[s] = [
                    ps_ot.tile([128, 512], f32, tag=f"ot{m}", name=f"pot{s}{m}")
                    for m in range(DC)
                ]

            # ================= emission ================================
            load_xt(0)
            load_wvt()
            load_xt(1)

            for b in range(BPC):
                gt_sb[b] = gpool.tile([128, DC * N], f32, tag="gt", name=f"gt{b}")
                v_sb[b] = vpool.tile([128, KC * D], f32, tag="v", name=f"v{b}")

            # ---- prologue: gt(0) k4-outer across 8 psum groups
            # (3x ws + 1x db for hh=0; 4x ot for hh=1), consuming xt
            # chunks in DMA-arrival order.  S(0,h0) and v(0) interleave
            # into the DMA-chase gaps so the PE never idles long enough
            # to drop its p-state.
            grp_order = [(m, 0) for m in range(DC)] + [(m, 1) for m in range(DC)]
            grp = {}
            for i, (m, hh) in enumerate(grp_order):
                if i < 3:
                    grp[(m, hh)] = ps_ws.tile(
                        [128, 512], f32, tag="ws", name=f"pg0_{m}{hh}"
                    )
                elif i == 3:
                    grp[(m, hh)] = ps_db.tile(
                        [128, 512], f32, tag="db", name=f"pg0_{m}{hh}"
                    )
                else:
                    grp[(m, hh)] = ps_ot.tile(
                        [128, 512], f32, tag=f"ot{i - 4}", name=f"pg0_{m}{hh}"
                    )

            def gt0_round(k4, hh):
                for m in range(DC):
                    nc.tensor.matmul(
                        grp[(m, hh)][:],
                        a_sb[k4][:, m * 128:(m + 1) * 128].bitcast(f32r),
                        xh[0][k4][hh][:].bitcast(f32r),
                        start=(k4 == 0), stop=(k4 == DC - 1),
                    )

            def gt0_evict(hh):
                # 256-wide halves split across ScalarE/VectorE so all four
                # chunks land ~1.4us after the psum groups stop
                for m in range(DC):
                    base = m * N + hh * 512
                    nc.scalar.copy(
                        gt_sb[0][:, base:base + 256].bitcast(f32r),
                        grp[(m, hh)][:, 0:256],
                    )
                    nc.vector.tensor_copy(
                        gt_sb[0][:, base + 256:base + 512].bitcast(f32r),
                        grp[(m, hh)][:, 256:512],
                    )

            for k4 in range(DC):
                gt0_round(k4, 0)
            gt0_evict(0)
            alloc_slot(0)
            # hh=1 rounds chase the xt0-hh1 chunk arrivals; S(0,h0,kc<4)
            # (which needs only the hh=0 gt chunks) fills the gaps
            for kind, i in (("r", 0), ("r", 1), ("s", 0), ("r", 2),
                            ("s", 1), ("r", 3), ("s", 2), ("s", 3)):
                if kind == "r":
                    gt0_round(i, 1)
                elif i == 3:
                    s_group(0, i, pool=ps_db, tag="db")
                else:
                    s_group(0, i)
            gt0_evict(1)
            # v(0) groups + the rest of S(0,h0)
            for kind, i in (("v", 0), ("v", 1), ("v", 2), ("s", 4),
                            ("v", 3), ("s", 5), ("v", 4), ("s", 6),
                            ("v", 5), ("s", 7), ("v", 6), ("v", 7)):
                if kind == "v":
                    v_group(0, i)
                else:
                    s_group(0, i)

            # ---- steady slots -----------------------------------------
            for s in range(1, NSLOT):
                b, h = s // 2, s % 2
                alloc_slot(s)
                if h == 1 and b + 2 < BPC:
                    load_xt(b + 2)

                for kc in range(KC):
                    if kc == 0 and s >= 2:
                        # free the ot psum banks first thing on Scalar/
                        # Vector so this slot's O groups are not blocked
                        evict_ot(s - 2, 0, "s")
                        evict_ot(s - 2, 1, "v")
                        evict_ot(s - 2, 2, "s")
                        evict_ot(s - 2, 3, "v")

                    # O leads at the last two steps so its psum stop (and
                    # the next slot's evictions) land earlier
                    if kc >= 6:
                        o_group(s - 1, kc)

                    s_group(s, kc)

                    if kc == 1:
                        # kc==1 keeps the norm matmul's counter-based waits
                        # clear of the slot-start Scalar/Vector backlog
                        norm_chain(s - 1)
                    if kc == 3 and s >= 2:
                        for m in range(DC):
                            mul_ot(s - 2, m)
                    if kc == 4 and s >= 2:
                        dma_out(s - 2, 0, "s")
                    if kc == 5 and s >= 2:
                        dma_out(s - 2, 1, "y")

                    if kc < 6:
                        o_group(s - 1, kc)

                    # gt(1) runs in slot 1 (xt(1) has fully landed by then)
                    if s == 1:
                        gt_group(1, kc % 4, kc // 4)
                        v_group(1, kc)
                    elif h == 0 and b >= 1 and b + 1 < BPC:
                        gt_group(b + 1, kc % 4, kc // 4)
                    elif h == 1 and s > 1 and b + 1 < BPC:
                        v_group(b + 1, kc)

            # ---- epilogue: tile L's O phase in two 256-col halves -----
            L = NSLOT - 1
            # free ot banks (tile L-2 was evicted in slot L; L-1 now)
            evict_ot(L - 1, 0, "s")
            evict_ot(L - 1, 1, "v")
            evict_ot(L - 1, 2, "s")
            evict_ot(L - 1, 3, "v")
            for kc in range(KC):
                o_group(L, kc, 0, 256)
                if kc == 1:
                    norm_chain(L, 0, 256, part="a")
                if kc == 2:
                    for m in range(DC):
                        mul_ot(L - 1, m)
                if kc == 3:
                    dma_out(L - 1, 0, "y")
                if kc == 4:
                    dma_out(L - 1, 1, "y")
            # half-a evictions free banks chunk by chunk for half-b
            for m in range(DC):
                evict_ot(L, m, "s" if m % 2 == 0 else "v", 0, 256)
            ot_b = [
                ps_ot.tile([128, 512], f32, tag=f"ot{m}", name=f"potb{m}")
                for m in range(DC)
            ]
            for kc in range(KC):
                o_group(L, kc, 256, 512, tiles=ot_b)
                if kc == 0:
                    norm_chain(L, 256, 512, part="b")
                if kc == 1:
                    for m in range(DC):
                        mul_ot(L, m, 0, 256)
                if kc == 3:
                    dma_out(L, 0, "s", 0, 256)
                if kc == 4:
                    dma_out(L, 1, "y", 0, 256)
            for m in range(DC):
                evict_ot(L, m, "s" if m % 2 == 0 else "v", 256, 512, tiles=ot_b)
            for m in range(DC):
                mul_ot(L, m, 256, 512)
                g, j = m // 2, m % 2
                q = nc.scalar if m % 2 == 0 else nc.sync
                dst = out_t[
                    L // 2, g * 256:(g + 1) * 256, (L % 2) * 512 + 256:(L % 2) * 512 + 512
                ].rearrange("(m p) q -> p m q", p=128)
                q.dma_start(
                    dst[:, j:j + 1, :],
                    otraw[L][:, m * 512 + 256:m * 512 + 512].rearrange(
                        "p (m q) -> p m q", m=1
                    ),
                )
    return nc


def _prepare_inputs(embeddings, Wq, Wk, Wv):
    xt_all = np.ascontiguousarray(embeddings.transpose(0, 2, 1)).astype(
        np.float32, copy=False
    )
    a_mat = (
        Wq.astype(np.float64).T @ Wk.astype(np.float64) / np.sqrt(float(D))
    ).astype(np.float32)
    wvt = np.ascontiguousarray(Wv.T).astype(np.float32, copy=False)
    ones_mat = np.ones((128, 128), np.float32)
    in_maps = []
    for i in range(NCORES):
        in_maps.append(
            {
                "xt": np.ascontiguousarray(xt_all[i * BPC:(i + 1) * BPC]),
                "a_mat": a_mat,
                "wvt": wvt,
                "ones_mat": ones_mat,
            }
        )
    return in_maps


def _get_nc():
    if "nc" not in _cache:
        nc = _build()
        _split_sync_waits(nc)
        _cache["nc"] = nc
    return _cache["nc"]


def _assemble(results):
    out = np.empty((B, N, D), np.float32)
    for i in range(NCORES):
        ot = results[i]["out_t"]  # [BPC, D, N]
        out[i * BPC:(i + 1) * BPC] = ot.transpose(0, 2, 1)
    return out


def kernel(embeddings, Wq, Wk, Wv):
    from concourse.bass_utils import run_bass_kernel_spmd

    embeddings = np.asarray(embeddings, dtype=np.float32)
    in_maps = _prepare_inputs(
        embeddings, np.asarray(Wq), np.asarray(Wk), np.asarray(Wv)
    )
    res = run_bass_kernel_spmd(_get_nc(), in_maps, list(range(NCORES)))
    return _assemble(res.results)
